# revision 30
# baseline (speedup 1.0000x reference)
"""Bass/Trainium2 kernel for nn_GCNN_61615600828570 (gated GCNN message passing).

Self-contained: hardcodes shapes/sharding. 8 NeuronCores, sharded as
(batch b, l-half h) — each core computes BOTH arc directions and all 10 edge
types for its 512 destination rows. A single pair AllGather exchanges the
transposed block-1 output x1^T between the two GCN blocks.

Key device-side choices:
- adjacency in fp8e4 (0/1 exact), all 20 (dir,type) slabs SBUF-resident
- arc matmuls in fp8e4 DoubleRow perf mode (2 k-slabs per instruction,
  0.5 cycles/row) with hi+lo quantization compensation of the projections
  (p = p_hi + p_lo, both e4m3; error ~1e-3)
- block-1 projections (x0 @ W + b) precomputed host-side straight into the
  fp8 hi/lo pair-tile layout; block-2 projections computed on device from
  the gathered x1^T, quantized to hi/lo on ACT/DVE.

kernel(**inputs) takes the FULL inputs (numpy, dtypes as in setup_inputs)
and returns the FULL (B, L, E) float32 output.
"""
import numpy as np
import ml_dtypes

import concourse.bass as bass
import concourse.mybir as mybir
import concourse.tile as tile
from concourse import bacc
from concourse.bass_utils import run_bass_kernel_spmd
from concourse.masks import make_identity

F32 = mybir.dt.float32
BF16 = mybir.dt.bfloat16
FP8 = mybir.dt.float8e4
BF = ml_dtypes.bfloat16
E4 = ml_dtypes.float8_e4m3fn
DR = mybir.MatmulPerfMode.DoubleRow

B, L, E, D = 4, 1024, 140, 140
NE, NU, NB = 10, 4, 2
ND = NU + 1
N1 = D + 1            # 141: D outputs + gate column
LH = L // 2           # 512 destination rows per core
LT = LH // 128        # 4 l-tiles per core
KT = L // 128         # 8 contraction k-tiles
KP = KT // 2          # 4 DoubleRow k-pair tiles
NCORES = 8
PAIRS = [[0, 1], [2, 3], [4, 5], [6, 7]]
NW = 2 * ND * N1      # 1410: in-sets 0..704, out-sets 705..1409
ET = [min(n, NU) for n in range(NE)]
# (dir, n) pair list; dir 0 = in-arcs (A), dir 1 = out-arcs (A^T)
DN = [(d, n) for n in range(NE) for d in (0, 1)]
P_CHUNKS = [(0, 512), (512, 1024), (1024, NW)]
XC = E * LH           # exchanged x1^T elements per core (140 rows x 512)

_NC = None
DEBUG_DUMPS = False


def _slab_col(d, n):
    return (ET[n] if d == 0 else ND + ET[n]) * N1


def _build(reps=1):
    nc = bacc.Bacc("TRN2", target_bir_lowering=False, debug=False,
                   num_devices=NCORES)

    # lhsT A slabs, one per (dir, n): [k=1024 global-m rows, 512 own-dest cols]
    am_d = nc.dram_tensor("am", [2 * NE, L, LH], FP8, kind="ExternalInput")
    # block-1 projections, host-quantized hi/lo pair tiles [128, KP*2*NW]
    p1h_d = nc.dram_tensor("p1h", [128, KP * 2 * NW], FP8, kind="ExternalInput")
    p1l_d = nc.dram_tensor("p1l", [128, KP * 2 * NW], FP8, kind="ExternalInput")
    x0_d = nc.dram_tensor("x0", [LH, E], F32, kind="ExternalInput")
    w0_d = nc.dram_tensor("w0", [128, NW], BF16, kind="ExternalInput")
    w1_d = nc.dram_tensor("w1", [16, NW], BF16, kind="ExternalInput")

    out_d = nc.dram_tensor("outp", [reps, LH, E], F32, kind="ExternalOutput")
    if DEBUG_DUMPS:
        x1o_d = nc.dram_tensor("x1o", [reps, 128, LT * E], BF16,
                               kind="ExternalOutput")
        x1fo_d = nc.dram_tensor("x1fo", [reps, 128, KT * E], BF16,
                                kind="ExternalOutput")
        p2ho_d = nc.dram_tensor("p2ho", [reps, 128, KT * NW], FP8,
                                kind="ExternalOutput")
        p2lo_d = nc.dram_tensor("p2lo", [reps, 128, KT * NW], FP8,
                                kind="ExternalOutput")
        acc2o_d = nc.dram_tensor("acc2o", [reps, 128, LT * D], F32,
                                 kind="ExternalOutput")


    # partition-major exchange buffers: straight [128, 560] SBUF<->DRAM copies
    cc_in = nc.dram_tensor("cc_in", [128, LT * E], BF16)
    cc_out = nc.dram_tensor("cc_out", [2, 128, LT * E], BF16)

    with tile.TileContext(nc) as tc:
        with (
            tc.tile_pool(name="cst", bufs=1) as cst,
            tc.tile_pool(name="amp", bufs=2 * NE + 2) as amp,
            tc.tile_pool(name="pp", bufs=1) as ppool,
            tc.tile_pool(name="xp", bufs=2) as xp,
            tc.tile_pool(name="wp", bufs=2) as wp,
            tc.tile_pool(name="gp", bufs=8) as gpool,
            tc.tile_pool(name="psarc", bufs=3, space="PSUM") as psarc,
            tc.tile_pool(name="psarc2", bufs=2, space="PSUM") as psarc2,
            tc.tile_pool(name="psmm", bufs=3, space="PSUM") as psmm,
        ):
            ident = cst.tile([128, 128], BF16)
            make_identity(nc, ident[:])

            am_view = am_d.ap().rearrange("a (j p) c -> a p j c", p=128)

            for rep in range(reps):
                # ---------------- phase 0: DMAs ----------------
                # single sync queue in exact consumption order: am0 first so
                # the PE can start, then the block-1 projections, then the
                # remaining A slabs; x0/w on the scalar queue (needed late)
                am = [amp.tile([128, KT * LH], FP8, tag="am", name=f"am{a}")
                      for a in range(len(DN))]

                def am_dma(a):
                    d, n = DN[a]
                    nc.sync.dma_start(
                        am[a][:].rearrange("p (j c) -> p j c", c=LH),
                        am_view[2 * n + d])

                am_dma(0)
                p1h = ppool.tile([128, KP * 2 * NW], FP8, tag="p1h", bufs=2)
                p1l = ppool.tile([128, KP * 2 * NW], FP8, tag="p1l", bufs=2)
                HW = KP * NW  # half the pair-tile columns (k-pairs 0,1)
                nc.sync.dma_start(p1h[:, 0:HW], p1h_d.ap()[:, 0:HW])
                nc.sync.dma_start(p1h[:, HW:], p1h_d.ap()[:, HW:])
                nc.sync.dma_start(p1l[:, 0:HW], p1l_d.ap()[:, 0:HW])
                nc.sync.dma_start(p1l[:, HW:], p1l_d.ap()[:, HW:])
                for a in range(1, len(DN)):
                    am_dma(a)
                x0 = xp.tile([128, LT * E], F32, tag="x0")
                nc.scalar.dma_start(x0[:].rearrange("p (t d) -> p t d", t=LT),
                                    x0_d.ap().rearrange("(t p) d -> p t d", p=128))
                w0 = wp.tile([128, NW], BF16, tag="w0")
                w1 = wp.tile([16, NW], BF16, tag="w1")
                nc.scalar.dma_start(w0[:], w0_d.ap())
                nc.scalar.dma_start(w1[:], w1_d.ap())

                def am_ap(a, i, lt):
                    # DoubleRow lhsT [128, 2, 128]: k-pair i, dest l-tile lt
                    return (am[a][:]
                            .rearrange("p (j c) -> p j c", c=LH)
                            [:, 2 * i:2 * i + 2, lt * 128:(lt + 1) * 128])

                def p_ap(ph, i, col):
                    # DoubleRow rhs [128, 2, 141]
                    return (ph[:]
                            .rearrange("p (i g c) -> p i g c", i=KP, g=2)
                            [:, i, :, col:col + N1])

                # -------- gated arc aggregation for one block --------
                def arc_block(ph, pl, acc, halves=(0, 1), a_outer=True):
                    stt_i = 0
                    loops = ([(a, half) for a, _ in enumerate(DN)
                              for half in halves] if a_outer else
                             [(a, half) for half in halves
                              for a, _ in enumerate(DN)])
                    for a, half in loops:
                        d, n = DN[a]
                        col = _slab_col(d, n)
                        if True:
                            pool = (psarc, psarc2)[(a + half) % 2]
                            arc = pool.tile([128, 512], F32, tag="arc",
                                            name="arc")
                            for lt in (2 * half, 2 * half + 1):
                                off = (lt % 2) * N1
                                for i in range(KP):
                                    nc.tensor.matmul(
                                        arc[:, off:off + N1], am_ap(a, i, lt),
                                        p_ap(ph, i, col),
                                        start=(i == 0), stop=False,
                                        perf_mode=DR)
                                for i in range(KP):
                                    nc.tensor.matmul(
                                        arc[:, off:off + N1], am_ap(a, i, lt),
                                        p_ap(pl, i, col),
                                        start=False, stop=(i == KP - 1),
                                        perf_mode=DR)
                            g_sb = gpool.tile([128, 2], F32, tag="g")
                            nc.scalar.activation(
                                g_sb[:], arc[:, D:D + N1 + 1:N1],
                                mybir.ActivationFunctionType.Sigmoid)
                            for lt in (2 * half, 2 * half + 1):
                                off = (lt % 2) * N1
                                stt_i += 1
                                if stt_i % 2 == 0:
                                    nc.vector.scalar_tensor_tensor(
                                        out=acc[:, lt * D:(lt + 1) * D],
                                        in0=arc[:, off:off + D],
                                        scalar=g_sb[:, lt % 2:lt % 2 + 1],
                                        in1=acc[:, lt * D:(lt + 1) * D],
                                        op0=mybir.AluOpType.mult,
                                        op1=mybir.AluOpType.add)
                                else:
                                    # ACT applies the gate (PSUM-legal), Pool
                                    # accumulates from SBUF
                                    garc = gpool.tile([128, D], F32,
                                                      tag="garc", bufs=4,
                                                      name="garc")
                                    nc.scalar.activation(
                                        garc[:], arc[:, off:off + D],
                                        mybir.ActivationFunctionType.Copy,
                                        scale=g_sb[:, lt % 2:lt % 2 + 1])
                                    nc.gpsimd.tensor_tensor(
                                        out=acc[:, lt * D:(lt + 1) * D],
                                        in0=acc[:, lt * D:(lt + 1) * D],
                                        in1=garc[:],
                                        op=mybir.AluOpType.add)

                # ---------------- block 1 ----------------
                acc = xp.tile([128, LT * D], F32, tag="acc")
                nc.gpsimd.memset(acc[:], 0.0)
                arc_block(p1h, p1l, acc)

                # x1 = relu(acc) + x0 ; exchange natural-layout x1 (bf16)
                x1b = xp.tile([128, LT * E], BF16, tag="x1b")
                nc.vector.scalar_tensor_tensor(
                    out=x1b[:], in0=acc[:], scalar=0.0, in1=x0[:],
                    op0=mybir.AluOpType.max, op1=mybir.AluOpType.add)

                # staging + collective + loads all on gpsimd: in-order queue
                # gives the DRAM read-after-write ordering the tile framework
                # does not track for dram tensors
                nc.gpsimd.dma_start(cc_in.ap(), x1b[:])
                nc.gpsimd.collective_compute(
                    "AllGather", mybir.AluOpType.bypass,
                    replica_groups=PAIRS,
                    ins=[cc_in.ap()], outs=[cc_out.ap()])
                # x1f col layout: global l-tile ltg at col ltg*E (h-major)
                x1f = xp.tile([128, KT * E], BF16, tag="x1f")
                nc.gpsimd.dma_start(x1f[:, 0:LT * E], cc_out.ap()[0])
                nc.gpsimd.dma_start(x1f[:, LT * E:], cc_out.ap()[1])

                # full x1^T in global order: [128|12] rows x 1024 cols (+ones)
                xta = xp.tile([128, L], BF16, tag="xta")
                xtb = xp.tile([32, L], BF16, tag="xtb")
                # aligned 32-row memset; transposes overwrite rows 0..11, so
                # row 12 keeps the 1.0 bias row (rows 13+ unused)
                nc.gpsimd.memset(xtb[0:32, :], 1.0)
                for lt in range(KT):
                    tp = psmm.tile([128, 512], BF16, tag="pmm", name="tp")
                    nc.tensor.transpose(tp[:, 0:128],
                                        x1f[:, lt * E:lt * E + 128], ident[:])
                    nc.scalar.copy(xta[:, lt * 128:(lt + 1) * 128],
                                   tp[:, 0:128])
                    tp2 = psmm.tile([128, 512], BF16, tag="pmm", name="tp2")
                    nc.tensor.transpose(tp2[0:E - 128, 0:128],
                                        x1f[:, lt * E + 128:lt * E + E],
                                        ident[:])
                    nc.scalar.copy(xtb[0:E - 128, lt * 128:(lt + 1) * 128],
                                   tp2[0:E - 128, 0:128])

                # ---------------- block-2 projections ----------------
                p2h = ppool.tile([128, KP * 2 * NW], FP8, tag="p2h")
                p2l = ppool.tile([128, KP * 2 * NW], FP8, tag="p2l")
                for m in range(KT):
                    for (c0, c1) in P_CHUNKS:
                        w = c1 - c0
                        pmm = psmm.tile([128, 512], F32, tag="pmm", name="pmm")
                        nc.tensor.matmul(pmm[:, 0:w],
                                         xta[:, m * 128:(m + 1) * 128],
                                         w0[:, c0:c1], start=True, stop=False)
                        nc.tensor.matmul(pmm[:, 0:w],
                                         xtb[0:13, m * 128:(m + 1) * 128],
                                         w1[0:13, c0:c1], start=False,
                                         stop=True)
                        dsth = p2h[:, m * NW + c0:m * NW + c1]
                        dstl = p2l[:, m * NW + c0:m * NW + c1]
                        nc.scalar.copy(dsth, pmm[:, 0:w])
                        nc.vector.tensor_tensor(
                            out=dstl, in0=pmm[:, 0:w], in1=dsth,
                            op=mybir.AluOpType.subtract)

                if DEBUG_DUMPS:
                    nc.sync.dma_start(x1o_d.ap()[rep], x1b[:])
                    nc.sync.dma_start(x1fo_d.ap()[rep], x1f[:])
                    nc.sync.dma_start(p2ho_d.ap()[rep], p2h[:])
                    nc.sync.dma_start(p2lo_d.ap()[rep], p2l[:])

                # ---------------- block 2 ----------------
                acc2 = xp.tile([128, LT * D], F32, tag="acc")
                nc.gpsimd.memset(acc2[:], 0.0)
                x2 = xp.tile([128, LT * E], F32, tag="x1")
                if DEBUG_DUMPS:
                    acc2_dump = acc2
                for half in (0, 1):
                    arc_block(p2h, p2l, acc2, halves=(half,), a_outer=False)
                    sl = slice(half * 2 * E, (half + 1) * 2 * E)
                    nc.vector.scalar_tensor_tensor(
                        out=x2[:, sl], in0=acc2[:, sl], scalar=0.0,
                        in1=x1b[:, sl],
                        op0=mybir.AluOpType.max, op1=mybir.AluOpType.add)
                    nc.sync.dma_start(
                        out_d.ap()[rep, half * 256:(half + 1) * 256].rearrange(
                            "(t p) d -> p t d", p=128),
                        x2[:, sl].rearrange("p (t d) -> p t d", t=2))
                    if DEBUG_DUMPS:
                        nc.sync.dma_start(
                            acc2o_d.ap()[rep][:, half * 2 * D:(half + 1) * 2 * D],
                            acc2[:, half * 2 * D:(half + 1) * 2 * D])

    nc.compile()
    return nc


def _get_nc():
    global _NC
    if _NC is None:
        _NC = _build()
    return _NC


def _prep_inputs(seq_repr, adj, W_in, b_in, W_out, b_out,
                 Wg_in, bg_in, Wg_out, bg_out):
    """Host-side sharding + layout prep for the 8 per-core input maps."""
    seq_repr = np.asarray(seq_repr, np.float32)
    adj = np.asarray(adj)
    W_in = np.asarray(W_in, np.float32); b_in = np.asarray(b_in, np.float32)
    W_out = np.asarray(W_out, np.float32); b_out = np.asarray(b_out, np.float32)
    Wg_in = np.asarray(Wg_in, np.float32); bg_in = np.asarray(bg_in, np.float32)
    Wg_out = np.asarray(Wg_out, np.float32); bg_out = np.asarray(bg_out, np.float32)

    # ---- block-2 weight slabs (shared by all cores) ----
    w0 = np.zeros((128, NW), np.float32)
    w1 = np.zeros((16, NW), np.float32)
    for d in range(2):
        Wd, bd = (W_in, b_in) if d == 0 else (W_out, b_out)
        Wgd, bgd = (Wg_in, bg_in) if d == 0 else (Wg_out, bg_out)
        for s in range(ND):
            c = (d * ND + s) * N1
            w0[:, c:c + D] = Wd[1, s][0:128]
            w1[0:12, c:c + D] = Wd[1, s][128:E]
            w1[12, c:c + D] = bd[1, s]
            w0[:, c + D] = Wgd[1, s][0:128, 0]
            w1[0:12, c + D] = Wgd[1, s][128:E, 0]
            w1[12, c + D] = bgd[1, s, 0]
    # x0.25: adjacency is scaled x4 (exact in fp8); keeps |p2| well under
    # the TRN2 fp8e4 saturation point (+-240)
    w0 = (0.25 * w0).astype(BF)
    w1 = (0.25 * w1).astype(BF)

    # ---- block-1 projections, host-computed, fp8 hi/lo pair tiles ----
    p1h_by_b, p1l_by_b = [], []
    for b in range(B):
        x = seq_repr[b]                       # (L, E)
        P = np.zeros((L, NW), np.float32)
        for d in range(2):
            Wd, bd = (W_in, b_in) if d == 0 else (W_out, b_out)
            Wgd, bgd = (Wg_in, bg_in) if d == 0 else (Wg_out, bg_out)
            for s in range(ND):
                c = (d * ND + s) * N1
                P[:, c:c + D] = x @ Wd[0, s] + bd[0, s]
                P[:, c + D] = x @ Wgd[0, s][:, 0] + bgd[0, s, 0]
        P *= 0.25            # compensates the x4-scaled adjacency
        Ph = P.astype(E4)
        Pl = (P - Ph.astype(np.float32)).astype(E4)
        # pair-tile layout [128, KT*NW]: t[p, m*NW + c] = P[m*128 + p, c]
        ph = np.ascontiguousarray(
            Ph.reshape(KT, 128, NW).transpose(1, 0, 2).reshape(128, KT * NW))
        pl = np.ascontiguousarray(
            Pl.reshape(KT, 128, NW).transpose(1, 0, 2).reshape(128, KT * NW))
        p1h_by_b.append(ph)
        p1l_by_b.append(pl)

    in_maps = []
    for c in range(NCORES):
        b, h = c // 2, c % 2
        sl = slice(h * LH, (h + 1) * LH)
        a = (adj[b] * 4).astype(np.int8)      # 0/4: exact in fp8e4
        am = np.empty((2 * NE, L, LH), E4)
        for n in range(NE):
            # in-arcs: lhsT[m, l] = A[l, m] for own dest rows l
            am[2 * n] = a[n][sl, :].T.astype(E4)
            # out-arcs: lhsT[m, l] = A^T[l, m] = A[m, l]
            am[2 * n + 1] = a[n][:, sl].astype(E4)
        in_maps.append({
            "am": am,
            "p1h": p1h_by_b[b], "p1l": p1l_by_b[b],
            "x0": np.ascontiguousarray(seq_repr[b][sl]),
            "w0": w0, "w1": w1,
        })
    return in_maps


def _combine(results):
    out = np.empty((B, L, E), np.float32)
    for b in range(B):
        out[b, 0:LH] = results[2 * b]["outp"][0]
        out[b, LH:L] = results[2 * b + 1]["outp"][0]
    return out


def run_on_hw(in_maps, trace=False, **kw):
    nc = _get_nc()
    res = run_bass_kernel_spmd(nc, in_maps, core_ids=list(range(NCORES)),
                               trace=trace, **kw)
    return res


def kernel(**inputs):
    in_maps = _prep_inputs(**inputs)
    res = run_on_hw(in_maps)
    return _combine(res.results)


# revision 35
# speedup vs baseline: 1.0042x; 1.0042x over previous
"""Bass/Trainium2 kernel for nn_GCNN_61615600828570 (gated GCNN message passing).

Self-contained: hardcodes shapes/sharding. 8 NeuronCores, sharded as
(batch b, l-half h) — each core computes BOTH arc directions and all 10 edge
types for its 512 destination rows. A single pair AllGather exchanges the
transposed block-1 output x1^T between the two GCN blocks.

Key device-side choices:
- adjacency in fp8e4 (0/1 exact), all 20 (dir,type) slabs SBUF-resident
- arc matmuls in fp8e4 DoubleRow perf mode (2 k-slabs per instruction,
  0.5 cycles/row) with hi+lo quantization compensation of the projections
  (p = p_hi + p_lo, both e4m3; error ~1e-3)
- block-1 projections (x0 @ W + b) precomputed host-side straight into the
  fp8 hi/lo pair-tile layout; block-2 projections computed on device from
  the gathered x1^T, quantized to hi/lo on ACT/DVE.

kernel(**inputs) takes the FULL inputs (numpy, dtypes as in setup_inputs)
and returns the FULL (B, L, E) float32 output.
"""
import numpy as np
import ml_dtypes

import concourse.bass as bass
import concourse.mybir as mybir
import concourse.tile as tile
from concourse import bacc
from concourse.bass_utils import run_bass_kernel_spmd
from concourse.masks import make_identity

F32 = mybir.dt.float32
BF16 = mybir.dt.bfloat16
FP8 = mybir.dt.float8e4
BF = ml_dtypes.bfloat16
E4 = ml_dtypes.float8_e4m3fn
DR = mybir.MatmulPerfMode.DoubleRow

B, L, E, D = 4, 1024, 140, 140
NE, NU, NB = 10, 4, 2
ND = NU + 1
N1 = D + 1            # 141: D outputs + gate column
LH = L // 2           # 512 destination rows per core
LT = LH // 128        # 4 l-tiles per core
KT = L // 128         # 8 contraction k-tiles
KP = KT // 2          # 4 DoubleRow k-pair tiles
NCORES = 8
PAIRS = [[0, 1], [2, 3], [4, 5], [6, 7]]
NW = 2 * ND * N1      # 1410: in-sets 0..704, out-sets 705..1409
ET = [min(n, NU) for n in range(NE)]
# (dir, n) pair list; dir 0 = in-arcs (A), dir 1 = out-arcs (A^T)
DN = [(d, n) for n in range(NE) for d in (0, 1)]
P_CHUNKS = [(0, 512), (512, 1024), (1024, NW)]
XC = E * LH           # exchanged x1^T elements per core (140 rows x 512)

_NC = None
DEBUG_DUMPS = False


def _slab_col(d, n):
    return (ET[n] if d == 0 else ND + ET[n]) * N1


def _build(reps=1):
    nc = bacc.Bacc("TRN2", target_bir_lowering=False, debug=False,
                   num_devices=NCORES)

    # lhsT A slabs, one per (dir, n): [k=1024 global-m rows, 512 own-dest cols]
    am_d = nc.dram_tensor("am", [2 * NE, L, LH], FP8, kind="ExternalInput")
    # block-1 projections, host-quantized hi/lo pair tiles [128, KP*2*NW]
    p1h_d = nc.dram_tensor("p1h", [128, KP * 2 * NW], FP8, kind="ExternalInput")
    p1l_d = nc.dram_tensor("p1l", [128, KP * 2 * NW], FP8, kind="ExternalInput")
    x0_d = nc.dram_tensor("x0", [LH, E], F32, kind="ExternalInput")
    w0_d = nc.dram_tensor("w0", [128, NW], BF16, kind="ExternalInput")
    w1_d = nc.dram_tensor("w1", [16, NW], BF16, kind="ExternalInput")

    out_d = nc.dram_tensor("outp", [reps, LH, E], F32, kind="ExternalOutput")
    if DEBUG_DUMPS:
        x1o_d = nc.dram_tensor("x1o", [reps, 128, LT * E], BF16,
                               kind="ExternalOutput")
        x1fo_d = nc.dram_tensor("x1fo", [reps, 128, KT * E], BF16,
                                kind="ExternalOutput")
        p2ho_d = nc.dram_tensor("p2ho", [reps, 128, KT * NW], FP8,
                                kind="ExternalOutput")
        p2lo_d = nc.dram_tensor("p2lo", [reps, 128, KT * NW], FP8,
                                kind="ExternalOutput")
        acc2o_d = nc.dram_tensor("acc2o", [reps, 128, LT * D], F32,
                                 kind="ExternalOutput")


    # partition-major exchange buffers: straight [128, 560] SBUF<->DRAM copies
    cc_in = nc.dram_tensor("cc_in", [128, LT * E], BF16)
    cc_out = nc.dram_tensor("cc_out", [2, 128, LT * E], BF16)

    with tile.TileContext(nc) as tc:
        with (
            tc.tile_pool(name="cst", bufs=1) as cst,
            tc.tile_pool(name="amp", bufs=2 * NE + 2) as amp,
            tc.tile_pool(name="pp", bufs=1) as ppool,
            tc.tile_pool(name="xp", bufs=2) as xp,
            tc.tile_pool(name="wp", bufs=2) as wp,
            tc.tile_pool(name="gp", bufs=8) as gpool,
            tc.tile_pool(name="psarc", bufs=3, space="PSUM") as psarc,
            tc.tile_pool(name="psarc2", bufs=2, space="PSUM") as psarc2,
            tc.tile_pool(name="psmm", bufs=3, space="PSUM") as psmm,
        ):
            ident = cst.tile([128, 128], BF16)
            make_identity(nc, ident[:])

            am_view = am_d.ap().rearrange("a (j p) c -> a p j c", p=128)

            for rep in range(reps):
                # ---------------- phase 0: DMAs ----------------
                # single sync queue in exact consumption order: am0 first so
                # the PE can start, then the block-1 projections, then the
                # remaining A slabs; x0/w on the scalar queue (needed late)
                am = [amp.tile([128, KT * LH], FP8, tag="am", name=f"am{a}")
                      for a in range(len(DN))]

                def am_dma(a):
                    d, n = DN[a]
                    nc.sync.dma_start(
                        am[a][:].rearrange("p (j c) -> p j c", c=LH),
                        am_view[2 * n + d])

                am_dma(0)
                p1h = ppool.tile([128, KP * 2 * NW], FP8, tag="p1h", bufs=2)
                p1l = ppool.tile([128, KP * 2 * NW], FP8, tag="p1l", bufs=2)
                HW = KP * NW  # half the pair-tile columns (k-pairs 0,1)
                nc.sync.dma_start(p1h[:, 0:HW], p1h_d.ap()[:, 0:HW])
                nc.sync.dma_start(p1h[:, HW:], p1h_d.ap()[:, HW:])
                nc.sync.dma_start(p1l[:, 0:HW], p1l_d.ap()[:, 0:HW])
                nc.sync.dma_start(p1l[:, HW:], p1l_d.ap()[:, HW:])
                for a in range(1, len(DN)):
                    am_dma(a)
                x0 = xp.tile([128, LT * E], F32, tag="x0")
                nc.scalar.dma_start(x0[:].rearrange("p (t d) -> p t d", t=LT),
                                    x0_d.ap().rearrange("(t p) d -> p t d", p=128))
                w0 = wp.tile([128, NW], BF16, tag="w0")
                w1 = wp.tile([16, NW], BF16, tag="w1")
                nc.scalar.dma_start(w0[:], w0_d.ap())
                nc.scalar.dma_start(w1[:], w1_d.ap())

                def am_ap(a, i, lt):
                    # DoubleRow lhsT [128, 2, 128]: k-pair i, dest l-tile lt
                    return (am[a][:]
                            .rearrange("p (j c) -> p j c", c=LH)
                            [:, 2 * i:2 * i + 2, lt * 128:(lt + 1) * 128])

                def p_ap(ph, i, col):
                    # DoubleRow rhs [128, 2, 141]
                    return (ph[:]
                            .rearrange("p (i g c) -> p i g c", i=KP, g=2)
                            [:, i, :, col:col + N1])

                # -------- gated arc aggregation for one block --------
                def arc_block(ph, pl, acc, halves=(0, 1), a_outer=True):
                    stt_i = 0
                    loops = ([(a, half) for a, _ in enumerate(DN)
                              for half in halves] if a_outer else
                             [(a, half) for half in halves
                              for a, _ in enumerate(DN)])
                    for a, half in loops:
                        d, n = DN[a]
                        col = _slab_col(d, n)
                        if True:
                            pool = (psarc, psarc2)[(a + half) % 2]
                            arc = pool.tile([128, 512], F32, tag="arc",
                                            name="arc")
                            for lt in (2 * half, 2 * half + 1):
                                off = (lt % 2) * N1
                                for i in range(KP):
                                    nc.tensor.matmul(
                                        arc[:, off:off + N1], am_ap(a, i, lt),
                                        p_ap(ph, i, col),
                                        start=(i == 0), stop=False,
                                        perf_mode=DR)
                                for i in range(KP):
                                    nc.tensor.matmul(
                                        arc[:, off:off + N1], am_ap(a, i, lt),
                                        p_ap(pl, i, col),
                                        start=False, stop=(i == KP - 1),
                                        perf_mode=DR)
                            g_sb = gpool.tile([128, 2], F32, tag="g")
                            nc.scalar.activation(
                                g_sb[:], arc[:, D:D + N1 + 1:N1],
                                mybir.ActivationFunctionType.Sigmoid)
                            for lt in (2 * half, 2 * half + 1):
                                off = (lt % 2) * N1
                                stt_i += 1
                                if stt_i % 2 == 0:
                                    nc.vector.scalar_tensor_tensor(
                                        out=acc[:, lt * D:(lt + 1) * D],
                                        in0=arc[:, off:off + D],
                                        scalar=g_sb[:, lt % 2:lt % 2 + 1],
                                        in1=acc[:, lt * D:(lt + 1) * D],
                                        op0=mybir.AluOpType.mult,
                                        op1=mybir.AluOpType.add)
                                else:
                                    # ACT applies the gate (PSUM-legal), Pool
                                    # accumulates from SBUF
                                    garc = gpool.tile([128, D], F32,
                                                      tag="garc", bufs=4,
                                                      name="garc")
                                    nc.scalar.activation(
                                        garc[:], arc[:, off:off + D],
                                        mybir.ActivationFunctionType.Copy,
                                        scale=g_sb[:, lt % 2:lt % 2 + 1])
                                    nc.gpsimd.tensor_tensor(
                                        out=acc[:, lt * D:(lt + 1) * D],
                                        in0=acc[:, lt * D:(lt + 1) * D],
                                        in1=garc[:],
                                        op=mybir.AluOpType.add)

                # ---------------- block 1 ----------------
                acc = xp.tile([128, LT * D], F32, tag="acc")
                nc.gpsimd.memset(acc[:], 0.0)
                arc_block(p1h, p1l, acc)

                # x1 = relu(acc) + x0 ; exchange natural-layout x1 (bf16)
                x1b = xp.tile([128, LT * E], BF16, tag="x1b")
                nc.vector.scalar_tensor_tensor(
                    out=x1b[:], in0=acc[:], scalar=0.0, in1=x0[:],
                    op0=mybir.AluOpType.max, op1=mybir.AluOpType.add)

                # staging + collective + loads all on gpsimd: in-order queue
                # gives the DRAM read-after-write ordering the tile framework
                # does not track for dram tensors
                nc.gpsimd.dma_start(cc_in.ap(), x1b[:])
                nc.gpsimd.collective_compute(
                    "AllGather", mybir.AluOpType.bypass,
                    replica_groups=PAIRS,
                    ins=[cc_in.ap()], outs=[cc_out.ap()])
                # x1f col layout: global l-tile ltg at col ltg*E (h-major)
                x1f = xp.tile([128, KT * E], BF16, tag="x1f")
                nc.gpsimd.dma_start(x1f[:, 0:LT * E], cc_out.ap()[0])
                nc.gpsimd.dma_start(x1f[:, LT * E:], cc_out.ap()[1])

                # full x1^T in global order: [128|12] rows x 1024 cols (+ones)
                xta = xp.tile([128, L], BF16, tag="xta")
                xtb = xp.tile([32, L], BF16, tag="xtb")
                # aligned 32-row memset; transposes overwrite rows 0..11, so
                # row 12 keeps the 1.0 bias row (rows 13+ unused)
                nc.gpsimd.memset(xtb[0:32, :], 1.0)
                for lt in range(KT):
                    tp = psmm.tile([128, 512], BF16, tag="pmm", name="tp")
                    nc.tensor.transpose(tp[:, 0:128],
                                        x1f[:, lt * E:lt * E + 128], ident[:])
                    nc.scalar.copy(xta[:, lt * 128:(lt + 1) * 128],
                                   tp[:, 0:128])
                    tp2 = psmm.tile([128, 512], BF16, tag="pmm", name="tp2")
                    nc.tensor.transpose(tp2[0:E - 128, 0:128],
                                        x1f[:, lt * E + 128:lt * E + E],
                                        ident[:])
                    nc.scalar.copy(xtb[0:E - 128, lt * 128:(lt + 1) * 128],
                                   tp2[0:E - 128, 0:128])

                # ---------------- block-2 projections ----------------
                p2h = ppool.tile([128, KP * 2 * NW], FP8, tag="p2h")
                p2l = ppool.tile([128, KP * 2 * NW], FP8, tag="p2l")
                for m in range(KT):
                    for (c0, c1) in P_CHUNKS:
                        w = c1 - c0
                        pmm = psmm.tile([128, 512], F32, tag="pmm", name="pmm")
                        nc.tensor.matmul(pmm[:, 0:w],
                                         xta[:, m * 128:(m + 1) * 128],
                                         w0[:, c0:c1], start=True, stop=False)
                        nc.tensor.matmul(pmm[:, 0:w],
                                         xtb[0:13, m * 128:(m + 1) * 128],
                                         w1[0:13, c0:c1], start=False,
                                         stop=True)
                        dsth = p2h[:, m * NW + c0:m * NW + c1]
                        dstl = p2l[:, m * NW + c0:m * NW + c1]
                        nc.scalar.copy(dsth, pmm[:, 0:w])
                        nc.vector.tensor_tensor(
                            out=dstl, in0=pmm[:, 0:w], in1=dsth,
                            op=mybir.AluOpType.subtract)

                if DEBUG_DUMPS:
                    nc.sync.dma_start(x1o_d.ap()[rep], x1b[:])
                    nc.sync.dma_start(x1fo_d.ap()[rep], x1f[:])
                    nc.sync.dma_start(p2ho_d.ap()[rep], p2h[:])
                    nc.sync.dma_start(p2lo_d.ap()[rep], p2l[:])

                # ---------------- block 2 ----------------
                acc2 = xp.tile([128, LT * D], F32, tag="acc")
                nc.gpsimd.memset(acc2[:], 0.0)
                x2 = xp.tile([128, LT * E], F32, tag="x1")
                if DEBUG_DUMPS:
                    acc2_dump = acc2
                arc_block(p2h, p2l, acc2)
                for half in (0, 1):
                    sl = slice(half * 2 * E, (half + 1) * 2 * E)
                    nc.vector.scalar_tensor_tensor(
                        out=x2[:, sl], in0=acc2[:, sl], scalar=0.0,
                        in1=x1b[:, sl],
                        op0=mybir.AluOpType.max, op1=mybir.AluOpType.add)
                    nc.sync.dma_start(
                        out_d.ap()[rep, half * 256:(half + 1) * 256].rearrange(
                            "(t p) d -> p t d", p=128),
                        x2[:, sl].rearrange("p (t d) -> p t d", t=2))
                    if DEBUG_DUMPS:
                        nc.sync.dma_start(
                            acc2o_d.ap()[rep][:, half * 2 * D:(half + 1) * 2 * D],
                            acc2[:, half * 2 * D:(half + 1) * 2 * D])

    nc.compile()
    return nc


def _get_nc():
    global _NC
    if _NC is None:
        _NC = _build()
    return _NC


def _prep_inputs(seq_repr, adj, W_in, b_in, W_out, b_out,
                 Wg_in, bg_in, Wg_out, bg_out):
    """Host-side sharding + layout prep for the 8 per-core input maps."""
    seq_repr = np.asarray(seq_repr, np.float32)
    adj = np.asarray(adj)
    W_in = np.asarray(W_in, np.float32); b_in = np.asarray(b_in, np.float32)
    W_out = np.asarray(W_out, np.float32); b_out = np.asarray(b_out, np.float32)
    Wg_in = np.asarray(Wg_in, np.float32); bg_in = np.asarray(bg_in, np.float32)
    Wg_out = np.asarray(Wg_out, np.float32); bg_out = np.asarray(bg_out, np.float32)

    # ---- block-2 weight slabs (shared by all cores) ----
    w0 = np.zeros((128, NW), np.float32)
    w1 = np.zeros((16, NW), np.float32)
    for d in range(2):
        Wd, bd = (W_in, b_in) if d == 0 else (W_out, b_out)
        Wgd, bgd = (Wg_in, bg_in) if d == 0 else (Wg_out, bg_out)
        for s in range(ND):
            c = (d * ND + s) * N1
            w0[:, c:c + D] = Wd[1, s][0:128]
            w1[0:12, c:c + D] = Wd[1, s][128:E]
            w1[12, c:c + D] = bd[1, s]
            w0[:, c + D] = Wgd[1, s][0:128, 0]
            w1[0:12, c + D] = Wgd[1, s][128:E, 0]
            w1[12, c + D] = bgd[1, s, 0]
    # x0.25: adjacency is scaled x4 (exact in fp8); keeps |p2| well under
    # the TRN2 fp8e4 saturation point (+-240)
    w0 = (0.25 * w0).astype(BF)
    w1 = (0.25 * w1).astype(BF)

    # ---- block-1 projections, host-computed, fp8 hi/lo pair tiles ----
    p1h_by_b, p1l_by_b = [], []
    for b in range(B):
        x = seq_repr[b]                       # (L, E)
        P = np.zeros((L, NW), np.float32)
        for d in range(2):
            Wd, bd = (W_in, b_in) if d == 0 else (W_out, b_out)
            Wgd, bgd = (Wg_in, bg_in) if d == 0 else (Wg_out, bg_out)
            for s in range(ND):
                c = (d * ND + s) * N1
                P[:, c:c + D] = x @ Wd[0, s] + bd[0, s]
                P[:, c + D] = x @ Wgd[0, s][:, 0] + bgd[0, s, 0]
        P *= 0.25            # compensates the x4-scaled adjacency
        Ph = P.astype(E4)
        Pl = (P - Ph.astype(np.float32)).astype(E4)
        # pair-tile layout [128, KT*NW]: t[p, m*NW + c] = P[m*128 + p, c]
        ph = np.ascontiguousarray(
            Ph.reshape(KT, 128, NW).transpose(1, 0, 2).reshape(128, KT * NW))
        pl = np.ascontiguousarray(
            Pl.reshape(KT, 128, NW).transpose(1, 0, 2).reshape(128, KT * NW))
        p1h_by_b.append(ph)
        p1l_by_b.append(pl)

    in_maps = []
    for c in range(NCORES):
        b, h = c // 2, c % 2
        sl = slice(h * LH, (h + 1) * LH)
        a = (adj[b] * 4).astype(np.int8)      # 0/4: exact in fp8e4
        am = np.empty((2 * NE, L, LH), E4)
        for n in range(NE):
            # in-arcs: lhsT[m, l] = A[l, m] for own dest rows l
            am[2 * n] = a[n][sl, :].T.astype(E4)
            # out-arcs: lhsT[m, l] = A^T[l, m] = A[m, l]
            am[2 * n + 1] = a[n][:, sl].astype(E4)
        in_maps.append({
            "am": am,
            "p1h": p1h_by_b[b], "p1l": p1l_by_b[b],
            "x0": np.ascontiguousarray(seq_repr[b][sl]),
            "w0": w0, "w1": w1,
        })
    return in_maps


def _combine(results):
    out = np.empty((B, L, E), np.float32)
    for b in range(B):
        out[b, 0:LH] = results[2 * b]["outp"][0]
        out[b, LH:L] = results[2 * b + 1]["outp"][0]
    return out


def run_on_hw(in_maps, trace=False, **kw):
    nc = _get_nc()
    res = run_bass_kernel_spmd(nc, in_maps, core_ids=list(range(NCORES)),
                               trace=trace, **kw)
    return res


def kernel(**inputs):
    in_maps = _prep_inputs(**inputs)
    res = run_on_hw(in_maps)
    return _combine(res.results)


# revision 41
# speedup vs baseline: 1.0117x; 1.0074x over previous
"""Bass/Trainium2 kernel for nn_GCNN_61615600828570 (gated GCNN message passing).

Self-contained: hardcodes shapes/sharding. 8 NeuronCores, sharded as
(batch b, l-half h) — each core computes BOTH arc directions and all 10 edge
types for its 512 destination rows. A single pair AllGather exchanges the
transposed block-1 output x1^T between the two GCN blocks.

Key device-side choices:
- adjacency in fp8e4 (0/1 exact), all 20 (dir,type) slabs SBUF-resident
- arc matmuls in fp8e4 DoubleRow perf mode (2 k-slabs per instruction,
  0.5 cycles/row) with hi+lo quantization compensation of the projections
  (p = p_hi + p_lo, both e4m3; error ~1e-3)
- block-1 projections (x0 @ W + b) precomputed host-side straight into the
  fp8 hi/lo pair-tile layout; block-2 projections computed on device from
  the gathered x1^T, quantized to hi/lo on ACT/DVE.

kernel(**inputs) takes the FULL inputs (numpy, dtypes as in setup_inputs)
and returns the FULL (B, L, E) float32 output.
"""
import numpy as np
import ml_dtypes

import concourse.bass as bass
import concourse.mybir as mybir
import concourse.tile as tile
from concourse import bacc
from concourse.bass_utils import run_bass_kernel_spmd
from concourse.masks import make_identity

F32 = mybir.dt.float32
BF16 = mybir.dt.bfloat16
FP8 = mybir.dt.float8e4
BF = ml_dtypes.bfloat16
E4 = ml_dtypes.float8_e4m3fn
DR = mybir.MatmulPerfMode.DoubleRow

B, L, E, D = 4, 1024, 140, 140
NE, NU, NB = 10, 4, 2
ND = NU + 1
N1 = D + 1            # 141: D outputs + gate column
LH = L // 2           # 512 destination rows per core
LT = LH // 128        # 4 l-tiles per core
KT = L // 128         # 8 contraction k-tiles
KP = KT // 2          # 4 DoubleRow k-pair tiles
NCORES = 8
PAIRS = [[0, 1], [2, 3], [4, 5], [6, 7]]
NW = 2 * ND * N1      # 1410: in-sets 0..704, out-sets 705..1409
ET = [min(n, NU) for n in range(NE)]
# (dir, n) pair list; dir 0 = in-arcs (A), dir 1 = out-arcs (A^T)
DN = [(d, n) for n in range(NE) for d in (0, 1)]
P_CHUNKS = [(0, 512), (512, 1024), (1024, NW)]
XC = E * LH           # exchanged x1^T elements per core (140 rows x 512)

_NC = None
DEBUG_DUMPS = False


def _slab_col(d, n):
    return (ET[n] if d == 0 else ND + ET[n]) * N1


def _build(reps=1):
    nc = bacc.Bacc("TRN2", target_bir_lowering=False, debug=False,
                   num_devices=NCORES)

    # lhsT A slabs, one per (dir, n): [k=1024 global-m rows, 512 own-dest cols]
    am_d = nc.dram_tensor("am", [2 * NE, L, LH], FP8, kind="ExternalInput")
    # block-1 projections, host-quantized hi/lo pair tiles [128, KP*2*NW]
    p1h_d = nc.dram_tensor("p1h", [128, KP * 2 * NW], FP8, kind="ExternalInput")
    p1l_d = nc.dram_tensor("p1l", [128, KP * 2 * NW], FP8, kind="ExternalInput")
    x0_d = nc.dram_tensor("x0", [LH, E], F32, kind="ExternalInput")
    w0_d = nc.dram_tensor("w0", [128, NW], BF16, kind="ExternalInput")
    w1_d = nc.dram_tensor("w1", [16, NW], BF16, kind="ExternalInput")

    out_d = nc.dram_tensor("outp", [reps, LH, E], F32, kind="ExternalOutput")
    if DEBUG_DUMPS:
        x1o_d = nc.dram_tensor("x1o", [reps, 128, LT * E], BF16,
                               kind="ExternalOutput")
        x1fo_d = nc.dram_tensor("x1fo", [reps, 128, KT * E], BF16,
                                kind="ExternalOutput")
        p2ho_d = nc.dram_tensor("p2ho", [reps, 128, KT * NW], FP8,
                                kind="ExternalOutput")
        p2lo_d = nc.dram_tensor("p2lo", [reps, 128, KT * NW], FP8,
                                kind="ExternalOutput")
        acc2o_d = nc.dram_tensor("acc2o", [reps, 128, LT * D], F32,
                                 kind="ExternalOutput")


    # partition-major exchange buffers: straight [128, 560] SBUF<->DRAM copies
    cc_in = nc.dram_tensor("cc_in", [128, LT * E], BF16)
    cc_out = nc.dram_tensor("cc_out", [2, 128, LT * E], BF16)

    with tile.TileContext(nc) as tc:
        with (
            tc.tile_pool(name="cst", bufs=1) as cst,
            tc.tile_pool(name="amp", bufs=2 * NE + 2) as amp,
            tc.tile_pool(name="pp", bufs=1) as ppool,
            tc.tile_pool(name="xp", bufs=2) as xp,
            tc.tile_pool(name="wp", bufs=2) as wp,
            tc.tile_pool(name="gp", bufs=24) as gpool,
            tc.tile_pool(name="psarc", bufs=3, space="PSUM") as psarc,
            tc.tile_pool(name="psarc2", bufs=2, space="PSUM") as psarc2,
            tc.tile_pool(name="psmm", bufs=3, space="PSUM") as psmm,
        ):
            ident = cst.tile([128, 128], BF16)
            make_identity(nc, ident[:])

            am_view = am_d.ap().rearrange("a (j p) c -> a p j c", p=128)

            for rep in range(reps):
                # ---------------- phase 0: DMAs ----------------
                # single sync queue in exact consumption order: am0 first so
                # the PE can start, then the block-1 projections, then the
                # remaining A slabs; x0/w on the scalar queue (needed late)
                am = [amp.tile([128, KT * LH], FP8, tag="am", name=f"am{a}")
                      for a in range(len(DN))]

                def am_dma(a):
                    d, n = DN[a]
                    nc.sync.dma_start(
                        am[a][:].rearrange("p (j c) -> p j c", c=LH),
                        am_view[2 * n + d])

                am_dma(0)
                p1h = ppool.tile([128, KP * 2 * NW], FP8, tag="p1h", bufs=2)
                p1l = ppool.tile([128, KP * 2 * NW], FP8, tag="p1l", bufs=2)
                HW = KP * NW  # half the pair-tile columns (k-pairs 0,1)
                nc.sync.dma_start(p1h[:, 0:HW], p1h_d.ap()[:, 0:HW])
                nc.sync.dma_start(p1h[:, HW:], p1h_d.ap()[:, HW:])
                nc.sync.dma_start(p1l[:, 0:HW], p1l_d.ap()[:, 0:HW])
                nc.sync.dma_start(p1l[:, HW:], p1l_d.ap()[:, HW:])
                for a in range(1, len(DN)):
                    am_dma(a)
                x0 = xp.tile([128, LT * E], F32, tag="x0")
                nc.scalar.dma_start(x0[:].rearrange("p (t d) -> p t d", t=LT),
                                    x0_d.ap().rearrange("(t p) d -> p t d", p=128))
                w0 = wp.tile([128, NW], BF16, tag="w0")
                w1 = wp.tile([16, NW], BF16, tag="w1")
                nc.scalar.dma_start(w0[:], w0_d.ap())
                nc.scalar.dma_start(w1[:], w1_d.ap())

                def am_ap(a, i, lt):
                    # DoubleRow lhsT [128, 2, 128]: k-pair i, dest l-tile lt
                    return (am[a][:]
                            .rearrange("p (j c) -> p j c", c=LH)
                            [:, 2 * i:2 * i + 2, lt * 128:(lt + 1) * 128])

                def p_ap(ph, i, col):
                    # DoubleRow rhs [128, 2, 141]
                    return (ph[:]
                            .rearrange("p (i g c) -> p i g c", i=KP, g=2)
                            [:, i, :, col:col + N1])

                # -------- gated arc aggregation for one block --------
                def arc_block(ph, pl, acc, halves=(0, 1), a_outer=True):
                    stt_i = 0
                    loops = ([(a, half) for a, _ in enumerate(DN)
                              for half in halves] if a_outer else
                             [(a, half) for half in halves
                              for a, _ in enumerate(DN)])
                    for a, half in loops:
                        d, n = DN[a]
                        col = _slab_col(d, n)
                        if True:
                            pool = (psarc, psarc2)[(a + half) % 2]
                            arc = pool.tile([128, 512], F32, tag="arc",
                                            name="arc")
                            for lt in (2 * half, 2 * half + 1):
                                off = (lt % 2) * N1
                                for i in range(KP):
                                    nc.tensor.matmul(
                                        arc[:, off:off + N1], am_ap(a, i, lt),
                                        p_ap(ph, i, col),
                                        start=(i == 0), stop=False,
                                        perf_mode=DR)
                                for i in range(KP):
                                    nc.tensor.matmul(
                                        arc[:, off:off + N1], am_ap(a, i, lt),
                                        p_ap(pl, i, col),
                                        start=False, stop=(i == KP - 1),
                                        perf_mode=DR)
                            g_sb = gpool.tile([128, 2], F32, tag="g")
                            nc.scalar.activation(
                                g_sb[:], arc[:, D:D + N1 + 1:N1],
                                mybir.ActivationFunctionType.Sigmoid)
                            for lt in (2 * half, 2 * half + 1):
                                off = (lt % 2) * N1
                                stt_i += 1
                                if stt_i % 2 == 0:
                                    nc.vector.scalar_tensor_tensor(
                                        out=acc[:, lt * D:(lt + 1) * D],
                                        in0=arc[:, off:off + D],
                                        scalar=g_sb[:, lt % 2:lt % 2 + 1],
                                        in1=acc[:, lt * D:(lt + 1) * D],
                                        op0=mybir.AluOpType.mult,
                                        op1=mybir.AluOpType.add)
                                else:
                                    # ACT applies the gate (PSUM-legal), Pool
                                    # accumulates from SBUF
                                    garc = gpool.tile([128, D], F32,
                                                      tag="garc", bufs=20,
                                                      name="garc")
                                    nc.scalar.activation(
                                        garc[:], arc[:, off:off + D],
                                        mybir.ActivationFunctionType.Copy,
                                        scale=g_sb[:, lt % 2:lt % 2 + 1])
                                    nc.gpsimd.tensor_tensor(
                                        out=acc[:, lt * D:(lt + 1) * D],
                                        in0=acc[:, lt * D:(lt + 1) * D],
                                        in1=garc[:],
                                        op=mybir.AluOpType.add)

                # ---------------- block 1 ----------------
                acc = xp.tile([128, LT * D], F32, tag="acc")
                nc.gpsimd.memset(acc[:], 0.0)
                arc_block(p1h, p1l, acc)

                # x1 = relu(acc) + x0 ; exchange natural-layout x1 (bf16)
                x1b = xp.tile([128, LT * E], BF16, tag="x1b")
                nc.vector.scalar_tensor_tensor(
                    out=x1b[:], in0=acc[:], scalar=0.0, in1=x0[:],
                    op0=mybir.AluOpType.max, op1=mybir.AluOpType.add)

                # staging + collective + loads all on gpsimd: in-order queue
                # gives the DRAM read-after-write ordering the tile framework
                # does not track for dram tensors
                nc.gpsimd.dma_start(cc_in.ap(), x1b[:])
                nc.gpsimd.collective_compute(
                    "AllGather", mybir.AluOpType.bypass,
                    replica_groups=PAIRS,
                    ins=[cc_in.ap()], outs=[cc_out.ap()])
                # x1f col layout: global l-tile ltg at col ltg*E (h-major)
                x1f = xp.tile([128, KT * E], BF16, tag="x1f")
                nc.gpsimd.dma_start(x1f[:, 0:LT * E], cc_out.ap()[0])
                nc.gpsimd.dma_start(x1f[:, LT * E:], cc_out.ap()[1])

                # full x1^T in global order: [128|12] rows x 1024 cols (+ones)
                xta = xp.tile([128, L], BF16, tag="xta")
                xtb = xp.tile([32, L], BF16, tag="xtb")
                # aligned 32-row memset; transposes overwrite rows 0..11, so
                # row 12 keeps the 1.0 bias row (rows 13+ unused)
                nc.gpsimd.memset(xtb[0:32, :], 1.0)
                for lt in range(KT):
                    tp = psmm.tile([128, 512], BF16, tag="pmm", name="tp")
                    nc.tensor.transpose(tp[:, 0:128],
                                        x1f[:, lt * E:lt * E + 128], ident[:])
                    nc.scalar.copy(xta[:, lt * 128:(lt + 1) * 128],
                                   tp[:, 0:128])
                    tp2 = psmm.tile([128, 512], BF16, tag="pmm", name="tp2")
                    nc.tensor.transpose(tp2[0:E - 128, 0:128],
                                        x1f[:, lt * E + 128:lt * E + E],
                                        ident[:])
                    nc.scalar.copy(xtb[0:E - 128, lt * 128:(lt + 1) * 128],
                                   tp2[0:E - 128, 0:128])

                # ---------------- block-2 projections ----------------
                p2h = ppool.tile([128, KP * 2 * NW], FP8, tag="p2h")
                p2l = ppool.tile([128, KP * 2 * NW], FP8, tag="p2l")
                for m in range(KT):
                    for (c0, c1) in P_CHUNKS:
                        w = c1 - c0
                        pmm = psmm.tile([128, 512], F32, tag="pmm", name="pmm")
                        nc.tensor.matmul(pmm[:, 0:w],
                                         xta[:, m * 128:(m + 1) * 128],
                                         w0[:, c0:c1], start=True, stop=False)
                        nc.tensor.matmul(pmm[:, 0:w],
                                         xtb[0:13, m * 128:(m + 1) * 128],
                                         w1[0:13, c0:c1], start=False,
                                         stop=True)
                        dsth = p2h[:, m * NW + c0:m * NW + c1]
                        dstl = p2l[:, m * NW + c0:m * NW + c1]
                        nc.scalar.copy(dsth, pmm[:, 0:w])
                        nc.vector.tensor_tensor(
                            out=dstl, in0=pmm[:, 0:w], in1=dsth,
                            op=mybir.AluOpType.subtract)

                if DEBUG_DUMPS:
                    nc.sync.dma_start(x1o_d.ap()[rep], x1b[:])
                    nc.sync.dma_start(x1fo_d.ap()[rep], x1f[:])
                    nc.sync.dma_start(p2ho_d.ap()[rep], p2h[:])
                    nc.sync.dma_start(p2lo_d.ap()[rep], p2l[:])

                # ---------------- block 2 ----------------
                acc2 = xp.tile([128, LT * D], F32, tag="acc")
                nc.gpsimd.memset(acc2[:], 0.0)
                x2 = xp.tile([128, LT * E], F32, tag="x1")
                if DEBUG_DUMPS:
                    acc2_dump = acc2
                arc_block(p2h, p2l, acc2)
                for half in (0, 1):
                    sl = slice(half * 2 * E, (half + 1) * 2 * E)
                    nc.vector.scalar_tensor_tensor(
                        out=x2[:, sl], in0=acc2[:, sl], scalar=0.0,
                        in1=x1b[:, sl],
                        op0=mybir.AluOpType.max, op1=mybir.AluOpType.add)
                    nc.sync.dma_start(
                        out_d.ap()[rep, half * 256:(half + 1) * 256].rearrange(
                            "(t p) d -> p t d", p=128),
                        x2[:, sl].rearrange("p (t d) -> p t d", t=2))
                    if DEBUG_DUMPS:
                        nc.sync.dma_start(
                            acc2o_d.ap()[rep][:, half * 2 * D:(half + 1) * 2 * D],
                            acc2[:, half * 2 * D:(half + 1) * 2 * D])

    nc.compile()
    return nc


def _get_nc():
    global _NC
    if _NC is None:
        _NC = _build()
    return _NC


def _prep_inputs(seq_repr, adj, W_in, b_in, W_out, b_out,
                 Wg_in, bg_in, Wg_out, bg_out):
    """Host-side sharding + layout prep for the 8 per-core input maps."""
    seq_repr = np.asarray(seq_repr, np.float32)
    adj = np.asarray(adj)
    W_in = np.asarray(W_in, np.float32); b_in = np.asarray(b_in, np.float32)
    W_out = np.asarray(W_out, np.float32); b_out = np.asarray(b_out, np.float32)
    Wg_in = np.asarray(Wg_in, np.float32); bg_in = np.asarray(bg_in, np.float32)
    Wg_out = np.asarray(Wg_out, np.float32); bg_out = np.asarray(bg_out, np.float32)

    # ---- block-2 weight slabs (shared by all cores) ----
    w0 = np.zeros((128, NW), np.float32)
    w1 = np.zeros((16, NW), np.float32)
    for d in range(2):
        Wd, bd = (W_in, b_in) if d == 0 else (W_out, b_out)
        Wgd, bgd = (Wg_in, bg_in) if d == 0 else (Wg_out, bg_out)
        for s in range(ND):
            c = (d * ND + s) * N1
            w0[:, c:c + D] = Wd[1, s][0:128]
            w1[0:12, c:c + D] = Wd[1, s][128:E]
            w1[12, c:c + D] = bd[1, s]
            w0[:, c + D] = Wgd[1, s][0:128, 0]
            w1[0:12, c + D] = Wgd[1, s][128:E, 0]
            w1[12, c + D] = bgd[1, s, 0]
    # x0.25: adjacency is scaled x4 (exact in fp8); keeps |p2| well under
    # the TRN2 fp8e4 saturation point (+-240)
    w0 = (0.25 * w0).astype(BF)
    w1 = (0.25 * w1).astype(BF)

    # ---- block-1 projections, host-computed, fp8 hi/lo pair tiles ----
    p1h_by_b, p1l_by_b = [], []
    for b in range(B):
        x = seq_repr[b]                       # (L, E)
        P = np.zeros((L, NW), np.float32)
        for d in range(2):
            Wd, bd = (W_in, b_in) if d == 0 else (W_out, b_out)
            Wgd, bgd = (Wg_in, bg_in) if d == 0 else (Wg_out, bg_out)
            for s in range(ND):
                c = (d * ND + s) * N1
                P[:, c:c + D] = x @ Wd[0, s] + bd[0, s]
                P[:, c + D] = x @ Wgd[0, s][:, 0] + bgd[0, s, 0]
        P *= 0.25            # compensates the x4-scaled adjacency
        Ph = P.astype(E4)
        Pl = (P - Ph.astype(np.float32)).astype(E4)
        # pair-tile layout [128, KT*NW]: t[p, m*NW + c] = P[m*128 + p, c]
        ph = np.ascontiguousarray(
            Ph.reshape(KT, 128, NW).transpose(1, 0, 2).reshape(128, KT * NW))
        pl = np.ascontiguousarray(
            Pl.reshape(KT, 128, NW).transpose(1, 0, 2).reshape(128, KT * NW))
        p1h_by_b.append(ph)
        p1l_by_b.append(pl)

    in_maps = []
    for c in range(NCORES):
        b, h = c // 2, c % 2
        sl = slice(h * LH, (h + 1) * LH)
        a = (adj[b] * 4).astype(np.int8)      # 0/4: exact in fp8e4
        am = np.empty((2 * NE, L, LH), E4)
        for n in range(NE):
            # in-arcs: lhsT[m, l] = A[l, m] for own dest rows l
            am[2 * n] = a[n][sl, :].T.astype(E4)
            # out-arcs: lhsT[m, l] = A^T[l, m] = A[m, l]
            am[2 * n + 1] = a[n][:, sl].astype(E4)
        in_maps.append({
            "am": am,
            "p1h": p1h_by_b[b], "p1l": p1l_by_b[b],
            "x0": np.ascontiguousarray(seq_repr[b][sl]),
            "w0": w0, "w1": w1,
        })
    return in_maps


def _combine(results):
    out = np.empty((B, L, E), np.float32)
    for b in range(B):
        out[b, 0:LH] = results[2 * b]["outp"][0]
        out[b, LH:L] = results[2 * b + 1]["outp"][0]
    return out


def run_on_hw(in_maps, trace=False, **kw):
    nc = _get_nc()
    res = run_bass_kernel_spmd(nc, in_maps, core_ids=list(range(NCORES)),
                               trace=trace, **kw)
    return res


def kernel(**inputs):
    in_maps = _prep_inputs(**inputs)
    res = run_on_hw(in_maps)
    return _combine(res.results)


# revision 49
# speedup vs baseline: 1.0180x; 1.0062x over previous
"""Bass/Trainium2 kernel for nn_GCNN_61615600828570 (gated GCNN message passing).

Self-contained: hardcodes shapes/sharding. 8 NeuronCores, sharded as
(batch b, l-half h) — each core computes BOTH arc directions and all 10 edge
types for its 512 destination rows. A single pair AllGather exchanges the
transposed block-1 output x1^T between the two GCN blocks.

Key device-side choices:
- adjacency in fp8e4 (0/1 exact), all 20 (dir,type) slabs SBUF-resident
- arc matmuls in fp8e4 DoubleRow perf mode (2 k-slabs per instruction,
  0.5 cycles/row) with hi+lo quantization compensation of the projections
  (p = p_hi + p_lo, both e4m3; error ~1e-3)
- block-1 projections (x0 @ W + b) precomputed host-side straight into the
  fp8 hi/lo pair-tile layout; block-2 projections computed on device from
  the gathered x1^T, quantized to hi/lo on ACT/DVE.

kernel(**inputs) takes the FULL inputs (numpy, dtypes as in setup_inputs)
and returns the FULL (B, L, E) float32 output.
"""
import numpy as np
import ml_dtypes

import concourse.bass as bass
import concourse.mybir as mybir
import concourse.tile as tile
from concourse import bacc
from concourse.bass_utils import run_bass_kernel_spmd
from concourse.masks import make_identity

F32 = mybir.dt.float32
BF16 = mybir.dt.bfloat16
FP8 = mybir.dt.float8e4
BF = ml_dtypes.bfloat16
E4 = ml_dtypes.float8_e4m3fn
DR = mybir.MatmulPerfMode.DoubleRow

B, L, E, D = 4, 1024, 140, 140
NE, NU, NB = 10, 4, 2
ND = NU + 1
N1 = D + 1            # 141: D outputs + gate column
LH = L // 2           # 512 destination rows per core
LT = LH // 128        # 4 l-tiles per core
KT = L // 128         # 8 contraction k-tiles
KP = KT // 2          # 4 DoubleRow k-pair tiles
NCORES = 8
PAIRS = [[0, 1], [2, 3], [4, 5], [6, 7]]
NW = 2 * ND * N1      # 1410: in-sets 0..704, out-sets 705..1409
ET = [min(n, NU) for n in range(NE)]
# (dir, n) pair list; dir 0 = in-arcs (A), dir 1 = out-arcs (A^T)
DN = [(d, n) for n in range(NE) for d in (0, 1)]
P_CHUNKS = [(0, 512), (512, 1024), (1024, NW)]
XC = E * LH           # exchanged x1^T elements per core (140 rows x 512)

_NC = None
DEBUG_DUMPS = False


def _slab_col(d, n):
    return (ET[n] if d == 0 else ND + ET[n]) * N1


def _build(reps=1):
    nc = bacc.Bacc("TRN2", target_bir_lowering=False, debug=False,
                   num_devices=NCORES)

    # lhsT A slabs, one per (dir, n): [k=1024 global-m rows, 512 own-dest cols]
    am_d = nc.dram_tensor("am", [2 * NE, L, LH], FP8, kind="ExternalInput")
    # block-1 projections, host-quantized hi/lo pair tiles [128, KP*2*NW]
    p1h_d = nc.dram_tensor("p1h", [128, KP * 2 * NW], FP8, kind="ExternalInput")
    p1l_d = nc.dram_tensor("p1l", [128, KP * 2 * NW], FP8, kind="ExternalInput")
    x0_d = nc.dram_tensor("x0", [LH, E], F32, kind="ExternalInput")
    w0_d = nc.dram_tensor("w0", [128, NW], BF16, kind="ExternalInput")
    w1_d = nc.dram_tensor("w1", [16, NW], BF16, kind="ExternalInput")

    out_d = nc.dram_tensor("outp", [reps, LH, E], F32, kind="ExternalOutput")
    if DEBUG_DUMPS:
        x1o_d = nc.dram_tensor("x1o", [reps, 128, LT * E], BF16,
                               kind="ExternalOutput")
        x1fo_d = nc.dram_tensor("x1fo", [reps, 128, KT * E], BF16,
                                kind="ExternalOutput")
        p2ho_d = nc.dram_tensor("p2ho", [reps, 128, KT * NW], FP8,
                                kind="ExternalOutput")
        p2lo_d = nc.dram_tensor("p2lo", [reps, 128, KT * NW], FP8,
                                kind="ExternalOutput")
        acc2o_d = nc.dram_tensor("acc2o", [reps, 128, LT * D], F32,
                                 kind="ExternalOutput")


    # partition-major exchange buffers: straight [128, 560] SBUF<->DRAM copies
    cc_in = nc.dram_tensor("cc_in", [128, LT * E], BF16)
    cc_out = nc.dram_tensor("cc_out", [2, 128, LT * E], BF16)

    with tile.TileContext(nc) as tc:
        with (
            tc.tile_pool(name="cst", bufs=1) as cst,
            tc.tile_pool(name="amp", bufs=2 * NE + 2) as amp,
            tc.tile_pool(name="pp", bufs=1) as ppool,
            tc.tile_pool(name="xp", bufs=2) as xp,
            tc.tile_pool(name="wp", bufs=2) as wp,
            tc.tile_pool(name="gp", bufs=24) as gpool,
            tc.tile_pool(name="psarc", bufs=3, space="PSUM") as psarc,
            tc.tile_pool(name="psarc2", bufs=2, space="PSUM") as psarc2,
            tc.tile_pool(name="psmm", bufs=3, space="PSUM") as psmm,
        ):
            ident = cst.tile([128, 128], BF16)
            make_identity(nc, ident[:])

            am_view = am_d.ap().rearrange("a (j p) c -> a p j c", p=128)

            for rep in range(reps):
                # ---------------- phase 0: DMAs ----------------
                # single sync queue in exact consumption order: am0 first so
                # the PE can start, then the block-1 projections, then the
                # remaining A slabs; x0/w on the scalar queue (needed late)
                am = [amp.tile([128, KT * LH], FP8, tag="am", name=f"am{a}")
                      for a in range(len(DN))]

                def am_dma(a):
                    d, n = DN[a]
                    nc.sync.dma_start(
                        am[a][:].rearrange("p (j c) -> p j c", c=LH),
                        am_view[2 * n + d])

                am_dma(0)
                p1h = ppool.tile([128, KP * 2 * NW], FP8, tag="p1h", bufs=2)
                p1l = ppool.tile([128, KP * 2 * NW], FP8, tag="p1l", bufs=2)
                HW = KP * NW  # half the pair-tile columns (k-pairs 0,1)
                nc.sync.dma_start(p1h[:, 0:HW], p1h_d.ap()[:, 0:HW])
                nc.sync.dma_start(p1h[:, HW:], p1h_d.ap()[:, HW:])
                nc.sync.dma_start(p1l[:, 0:HW], p1l_d.ap()[:, 0:HW])
                nc.sync.dma_start(p1l[:, HW:], p1l_d.ap()[:, HW:])
                for a in range(1, len(DN)):
                    am_dma(a)
                x0 = xp.tile([128, LT * E], F32, tag="x0")
                nc.scalar.dma_start(x0[:].rearrange("p (t d) -> p t d", t=LT),
                                    x0_d.ap().rearrange("(t p) d -> p t d", p=128))
                w0 = wp.tile([128, NW], BF16, tag="w0")
                w1 = wp.tile([16, NW], BF16, tag="w1")
                nc.scalar.dma_start(w0[:], w0_d.ap())
                nc.scalar.dma_start(w1[:], w1_d.ap())

                def am_ap(a, i, lt):
                    # DoubleRow lhsT [128, 2, 128]: k-pair i, dest l-tile lt
                    return (am[a][:]
                            .rearrange("p (j c) -> p j c", c=LH)
                            [:, 2 * i:2 * i + 2, lt * 128:(lt + 1) * 128])

                def p_ap(ph, i, col):
                    # DoubleRow rhs [128, 2, 141]
                    return (ph[:]
                            .rearrange("p (i g c) -> p i g c", i=KP, g=2)
                            [:, i, :, col:col + N1])

                # -------- gated arc aggregation for one block --------
                def arc_block(ph, pl, acc, halves=(0, 1), a_outer=True):
                    stt_i = 0
                    loops = ([(a, half) for a, _ in enumerate(DN)
                              for half in halves] if a_outer else
                             [(a, half) for half in halves
                              for a, _ in enumerate(DN)])
                    for a, half in loops:
                        d, n = DN[a]
                        col = _slab_col(d, n)
                        if True:
                            pool = (psarc, psarc2)[(a + half) % 2]
                            arc = pool.tile([128, 512], F32, tag="arc",
                                            name="arc")
                            for lt in (2 * half, 2 * half + 1):
                                off = (lt % 2) * N1
                                for i in range(KP):
                                    nc.tensor.matmul(
                                        arc[:, off:off + N1], am_ap(a, i, lt),
                                        p_ap(ph, i, col),
                                        start=(i == 0), stop=False,
                                        perf_mode=DR)
                                for i in range(KP):
                                    nc.tensor.matmul(
                                        arc[:, off:off + N1], am_ap(a, i, lt),
                                        p_ap(pl, i, col),
                                        start=False, stop=(i == KP - 1),
                                        perf_mode=DR)
                            g_sb = gpool.tile([128, 2], F32, tag="g")
                            nc.scalar.activation(
                                g_sb[:], arc[:, D:D + N1 + 1:N1],
                                mybir.ActivationFunctionType.Sigmoid)
                            for lt in (2 * half, 2 * half + 1):
                                off = (lt % 2) * N1
                                stt_i += 1
                                if stt_i % 2 == 0:
                                    nc.vector.scalar_tensor_tensor(
                                        out=acc[:, lt * D:(lt + 1) * D],
                                        in0=arc[:, off:off + D],
                                        scalar=g_sb[:, lt % 2:lt % 2 + 1],
                                        in1=acc[:, lt * D:(lt + 1) * D],
                                        op0=mybir.AluOpType.mult,
                                        op1=mybir.AluOpType.add)
                                else:
                                    # ACT applies the gate (PSUM-legal), Pool
                                    # accumulates from SBUF
                                    garc = gpool.tile([128, D], F32,
                                                      tag="garc", bufs=20,
                                                      name="garc")
                                    nc.scalar.activation(
                                        garc[:], arc[:, off:off + D],
                                        mybir.ActivationFunctionType.Copy,
                                        scale=g_sb[:, lt % 2:lt % 2 + 1])
                                    nc.gpsimd.tensor_tensor(
                                        out=acc[:, lt * D:(lt + 1) * D],
                                        in0=acc[:, lt * D:(lt + 1) * D],
                                        in1=garc[:],
                                        op=mybir.AluOpType.add)

                # ---------------- block 1 ----------------
                acc = xp.tile([128, LT * D], F32, tag="acc")
                nc.gpsimd.memset(acc[:], 0.0)
                arc_block(p1h, p1l, acc)

                # x1 = relu(acc) + x0 ; exchange natural-layout x1 (bf16)
                x1b = xp.tile([128, LT * E], BF16, tag="x1b")
                nc.vector.scalar_tensor_tensor(
                    out=x1b[:], in0=acc[:], scalar=0.0, in1=x0[:],
                    op0=mybir.AluOpType.max, op1=mybir.AluOpType.add)

                # staging + collective + loads all on gpsimd: in-order queue
                # gives the DRAM read-after-write ordering the tile framework
                # does not track for dram tensors
                nc.gpsimd.dma_start(cc_in.ap(), x1b[:])
                nc.gpsimd.collective_compute(
                    "AllGather", mybir.AluOpType.bypass,
                    replica_groups=PAIRS,
                    ins=[cc_in.ap()], outs=[cc_out.ap()])
                # x1f col layout: global l-tile ltg at col ltg*E (h-major)
                x1f = xp.tile([128, KT * E], BF16, tag="x1f")
                nc.gpsimd.dma_start(x1f[:, 0:LT * E], cc_out.ap()[0])
                nc.gpsimd.dma_start(x1f[:, LT * E:], cc_out.ap()[1])

                # full x1^T in global order: [128|12] rows x 1024 cols (+ones)
                xta = xp.tile([128, L], BF16, tag="xta")
                xtb = xp.tile([32, L], BF16, tag="xtb")
                # aligned 32-row memset; transposes overwrite rows 0..11, so
                # row 12 keeps the 1.0 bias row (rows 13+ unused)
                nc.gpsimd.memset(xtb[0:32, :], 1.0)
                for lt in range(KT):
                    tp = psmm.tile([128, 512], BF16, tag="pmm", name="tp")
                    nc.tensor.transpose(tp[:, 0:128],
                                        x1f[:, lt * E:lt * E + 128], ident[:])
                    if lt % 2 == 0:
                        nc.scalar.copy(xta[:, lt * 128:(lt + 1) * 128],
                                       tp[:, 0:128])
                    else:
                        nc.vector.tensor_copy(
                            xta[:, lt * 128:(lt + 1) * 128], tp[:, 0:128])
                    tp2 = psmm.tile([128, 512], BF16, tag="pmm", name="tp2")
                    nc.tensor.transpose(tp2[0:E - 128, 0:128],
                                        x1f[:, lt * E + 128:lt * E + E],
                                        ident[:])
                    if lt % 2 == 0:
                        nc.vector.tensor_copy(
                            xtb[0:E - 128, lt * 128:(lt + 1) * 128],
                            tp2[0:E - 128, 0:128])
                    else:
                        nc.scalar.copy(
                            xtb[0:E - 128, lt * 128:(lt + 1) * 128],
                            tp2[0:E - 128, 0:128])

                # ---------------- block-2 projections ----------------
                p2h = ppool.tile([128, KP * 2 * NW], FP8, tag="p2h")
                p2l = ppool.tile([128, KP * 2 * NW], FP8, tag="p2l")
                for m in range(KT):
                    for (c0, c1) in P_CHUNKS:
                        w = c1 - c0
                        pmm = psmm.tile([128, 512], F32, tag="pmm", name="pmm")
                        nc.tensor.matmul(pmm[:, 0:w],
                                         xta[:, m * 128:(m + 1) * 128],
                                         w0[:, c0:c1], start=True, stop=False)
                        nc.tensor.matmul(pmm[:, 0:w],
                                         xtb[0:13, m * 128:(m + 1) * 128],
                                         w1[0:13, c0:c1], start=False,
                                         stop=True)
                        dsth = p2h[:, m * NW + c0:m * NW + c1]
                        dstl = p2l[:, m * NW + c0:m * NW + c1]
                        nc.scalar.copy(dsth, pmm[:, 0:w])
                        nc.vector.tensor_tensor(
                            out=dstl, in0=pmm[:, 0:w], in1=dsth,
                            op=mybir.AluOpType.subtract)

                if DEBUG_DUMPS:
                    nc.sync.dma_start(x1o_d.ap()[rep], x1b[:])
                    nc.sync.dma_start(x1fo_d.ap()[rep], x1f[:])
                    nc.sync.dma_start(p2ho_d.ap()[rep], p2h[:])
                    nc.sync.dma_start(p2lo_d.ap()[rep], p2l[:])

                # ---------------- block 2 ----------------
                acc2 = xp.tile([128, LT * D], F32, tag="acc")
                nc.gpsimd.memset(acc2[:], 0.0)
                x2 = xp.tile([128, LT * E], F32, tag="x1")
                if DEBUG_DUMPS:
                    acc2_dump = acc2
                arc_block(p2h, p2l, acc2)
                for half in (0, 1):
                    sl = slice(half * 2 * E, (half + 1) * 2 * E)
                    nc.vector.scalar_tensor_tensor(
                        out=x2[:, sl], in0=acc2[:, sl], scalar=0.0,
                        in1=x1b[:, sl],
                        op0=mybir.AluOpType.max, op1=mybir.AluOpType.add)
                    nc.sync.dma_start(
                        out_d.ap()[rep, half * 256:(half + 1) * 256].rearrange(
                            "(t p) d -> p t d", p=128),
                        x2[:, sl].rearrange("p (t d) -> p t d", t=2))
                    if DEBUG_DUMPS:
                        nc.sync.dma_start(
                            acc2o_d.ap()[rep][:, half * 2 * D:(half + 1) * 2 * D],
                            acc2[:, half * 2 * D:(half + 1) * 2 * D])

    nc.compile()
    return nc


def _get_nc():
    global _NC
    if _NC is None:
        _NC = _build()
    return _NC


def _prep_inputs(seq_repr, adj, W_in, b_in, W_out, b_out,
                 Wg_in, bg_in, Wg_out, bg_out):
    """Host-side sharding + layout prep for the 8 per-core input maps."""
    seq_repr = np.asarray(seq_repr, np.float32)
    adj = np.asarray(adj)
    W_in = np.asarray(W_in, np.float32); b_in = np.asarray(b_in, np.float32)
    W_out = np.asarray(W_out, np.float32); b_out = np.asarray(b_out, np.float32)
    Wg_in = np.asarray(Wg_in, np.float32); bg_in = np.asarray(bg_in, np.float32)
    Wg_out = np.asarray(Wg_out, np.float32); bg_out = np.asarray(bg_out, np.float32)

    # ---- block-2 weight slabs (shared by all cores) ----
    w0 = np.zeros((128, NW), np.float32)
    w1 = np.zeros((16, NW), np.float32)
    for d in range(2):
        Wd, bd = (W_in, b_in) if d == 0 else (W_out, b_out)
        Wgd, bgd = (Wg_in, bg_in) if d == 0 else (Wg_out, bg_out)
        for s in range(ND):
            c = (d * ND + s) * N1
            w0[:, c:c + D] = Wd[1, s][0:128]
            w1[0:12, c:c + D] = Wd[1, s][128:E]
            w1[12, c:c + D] = bd[1, s]
            w0[:, c + D] = Wgd[1, s][0:128, 0]
            w1[0:12, c + D] = Wgd[1, s][128:E, 0]
            w1[12, c + D] = bgd[1, s, 0]
    # x0.25: adjacency is scaled x4 (exact in fp8); keeps |p2| well under
    # the TRN2 fp8e4 saturation point (+-240)
    w0 = (0.25 * w0).astype(BF)
    w1 = (0.25 * w1).astype(BF)

    # ---- block-1 projections, host-computed, fp8 hi/lo pair tiles ----
    p1h_by_b, p1l_by_b = [], []
    for b in range(B):
        x = seq_repr[b]                       # (L, E)
        P = np.zeros((L, NW), np.float32)
        for d in range(2):
            Wd, bd = (W_in, b_in) if d == 0 else (W_out, b_out)
            Wgd, bgd = (Wg_in, bg_in) if d == 0 else (Wg_out, bg_out)
            for s in range(ND):
                c = (d * ND + s) * N1
                P[:, c:c + D] = x @ Wd[0, s] + bd[0, s]
                P[:, c + D] = x @ Wgd[0, s][:, 0] + bgd[0, s, 0]
        P *= 0.25            # compensates the x4-scaled adjacency
        Ph = P.astype(E4)
        Pl = (P - Ph.astype(np.float32)).astype(E4)
        # pair-tile layout [128, KT*NW]: t[p, m*NW + c] = P[m*128 + p, c]
        ph = np.ascontiguousarray(
            Ph.reshape(KT, 128, NW).transpose(1, 0, 2).reshape(128, KT * NW))
        pl = np.ascontiguousarray(
            Pl.reshape(KT, 128, NW).transpose(1, 0, 2).reshape(128, KT * NW))
        p1h_by_b.append(ph)
        p1l_by_b.append(pl)

    in_maps = []
    for c in range(NCORES):
        b, h = c // 2, c % 2
        sl = slice(h * LH, (h + 1) * LH)
        a = (adj[b] * 4).astype(np.int8)      # 0/4: exact in fp8e4
        am = np.empty((2 * NE, L, LH), E4)
        for n in range(NE):
            # in-arcs: lhsT[m, l] = A[l, m] for own dest rows l
            am[2 * n] = a[n][sl, :].T.astype(E4)
            # out-arcs: lhsT[m, l] = A^T[l, m] = A[m, l]
            am[2 * n + 1] = a[n][:, sl].astype(E4)
        in_maps.append({
            "am": am,
            "p1h": p1h_by_b[b], "p1l": p1l_by_b[b],
            "x0": np.ascontiguousarray(seq_repr[b][sl]),
            "w0": w0, "w1": w1,
        })
    return in_maps


def _combine(results):
    out = np.empty((B, L, E), np.float32)
    for b in range(B):
        out[b, 0:LH] = results[2 * b]["outp"][0]
        out[b, LH:L] = results[2 * b + 1]["outp"][0]
    return out


def run_on_hw(in_maps, trace=False, **kw):
    nc = _get_nc()
    res = run_bass_kernel_spmd(nc, in_maps, core_ids=list(range(NCORES)),
                               trace=trace, **kw)
    return res


def kernel(**inputs):
    in_maps = _prep_inputs(**inputs)
    res = run_on_hw(in_maps)
    return _combine(res.results)


# revision 54
# speedup vs baseline: 1.0218x; 1.0038x over previous
"""Bass/Trainium2 kernel for nn_GCNN_61615600828570 (gated GCNN message passing).

Self-contained: hardcodes shapes/sharding. 8 NeuronCores, sharded as
(batch b, l-half h) — each core computes BOTH arc directions and all 10 edge
types for its 512 destination rows. A single pair AllGather exchanges the
transposed block-1 output x1^T between the two GCN blocks.

Key device-side choices:
- adjacency in fp8e4 (0/1 exact), all 20 (dir,type) slabs SBUF-resident
- arc matmuls in fp8e4 DoubleRow perf mode (2 k-slabs per instruction,
  0.5 cycles/row) with hi+lo quantization compensation of the projections
  (p = p_hi + p_lo, both e4m3; error ~1e-3)
- block-1 projections (x0 @ W + b) precomputed host-side straight into the
  fp8 hi/lo pair-tile layout; block-2 projections computed on device from
  the gathered x1^T, quantized to hi/lo on ACT/DVE.

kernel(**inputs) takes the FULL inputs (numpy, dtypes as in setup_inputs)
and returns the FULL (B, L, E) float32 output.
"""
import numpy as np
import ml_dtypes

import concourse.bass as bass
import concourse.mybir as mybir
import concourse.tile as tile
from concourse import bacc
from concourse.bass_utils import run_bass_kernel_spmd
from concourse.masks import make_identity

F32 = mybir.dt.float32
BF16 = mybir.dt.bfloat16
FP8 = mybir.dt.float8e4
BF = ml_dtypes.bfloat16
E4 = ml_dtypes.float8_e4m3fn
DR = mybir.MatmulPerfMode.DoubleRow

B, L, E, D = 4, 1024, 140, 140
NE, NU, NB = 10, 4, 2
ND = NU + 1
N1 = D + 1            # 141: D outputs + gate column
LH = L // 2           # 512 destination rows per core
LT = LH // 128        # 4 l-tiles per core
KT = L // 128         # 8 contraction k-tiles
KP = KT // 2          # 4 DoubleRow k-pair tiles
NCORES = 8
PAIRS = [[0, 1], [2, 3], [4, 5], [6, 7]]
NW = 2 * ND * N1      # 1410: in-sets 0..704, out-sets 705..1409
ET = [min(n, NU) for n in range(NE)]
# (dir, n) pair list; dir 0 = in-arcs (A), dir 1 = out-arcs (A^T)
DN = [(d, n) for n in range(NE) for d in (0, 1)]
P_CHUNKS = [(0, 512), (512, 1024), (1024, NW)]
XC = E * LH           # exchanged x1^T elements per core (140 rows x 512)

_NC = None
DEBUG_DUMPS = False


def _slab_col(d, n):
    return (ET[n] if d == 0 else ND + ET[n]) * N1


def _build(reps=1):
    nc = bacc.Bacc("TRN2", target_bir_lowering=False, debug=False,
                   num_devices=NCORES)

    # lhsT A slabs, one per (dir, n): [k=1024 global-m rows, 512 own-dest cols]
    am_d = nc.dram_tensor("am", [2 * NE, L, LH], FP8, kind="ExternalInput")
    # block-1 projections, host-quantized hi/lo pair tiles [128, KP*2*NW]
    p1h_d = nc.dram_tensor("p1h", [128, KP * 2 * NW], FP8, kind="ExternalInput")
    p1l_d = nc.dram_tensor("p1l", [128, KP * 2 * NW], FP8, kind="ExternalInput")
    x0_d = nc.dram_tensor("x0", [LH, E], F32, kind="ExternalInput")
    w0_d = nc.dram_tensor("w0", [128, NW], BF16, kind="ExternalInput")
    w1_d = nc.dram_tensor("w1", [16, NW], BF16, kind="ExternalInput")

    out_d = nc.dram_tensor("outp", [reps, LH, E], F32, kind="ExternalOutput")
    if DEBUG_DUMPS:
        x1o_d = nc.dram_tensor("x1o", [reps, 128, LT * E], BF16,
                               kind="ExternalOutput")
        x1fo_d = nc.dram_tensor("x1fo", [reps, 128, KT * E], BF16,
                                kind="ExternalOutput")
        p2ho_d = nc.dram_tensor("p2ho", [reps, 128, KT * NW], FP8,
                                kind="ExternalOutput")
        p2lo_d = nc.dram_tensor("p2lo", [reps, 128, KT * NW], FP8,
                                kind="ExternalOutput")
        acc2o_d = nc.dram_tensor("acc2o", [reps, 128, LT * D], F32,
                                 kind="ExternalOutput")


    # partition-major exchange buffers: straight [128, 560] SBUF<->DRAM copies
    cc_in = nc.dram_tensor("cc_in", [128, LT * E], BF16)
    cc_out = nc.dram_tensor("cc_out", [2, 128, LT * E], BF16)

    with tile.TileContext(nc) as tc:
        with (
            tc.tile_pool(name="cst", bufs=1) as cst,
            tc.tile_pool(name="amp", bufs=2 * NE) as amp,
            tc.tile_pool(name="pp", bufs=1) as ppool,
            tc.tile_pool(name="xp", bufs=2) as xp,
            tc.tile_pool(name="wp", bufs=2) as wp,
            tc.tile_pool(name="gp", bufs=24) as gpool,
            tc.tile_pool(name="psarc", bufs=3, space="PSUM") as psarc,
            tc.tile_pool(name="psarc2", bufs=2, space="PSUM") as psarc2,
            tc.tile_pool(name="psmm", bufs=3, space="PSUM") as psmm,
        ):
            ident = cst.tile([128, 128], BF16)
            make_identity(nc, ident[:])

            am_view = am_d.ap().rearrange("a (j p) c -> a p j c", p=128)

            for rep in range(reps):
                # ---------------- phase 0: DMAs ----------------
                # single sync queue in exact consumption order: am0 first so
                # the PE can start, then the block-1 projections, then the
                # remaining A slabs; x0/w on the scalar queue (needed late)
                am = [amp.tile([128, KT * LH], FP8, tag="am", name=f"am{a}")
                      for a in range(len(DN))]

                def am_dma(a):
                    d, n = DN[a]
                    nc.sync.dma_start(
                        am[a][:].rearrange("p (j c) -> p j c", c=LH),
                        am_view[2 * n + d])

                am_dma(0)
                p1h = ppool.tile([128, KP * 2 * NW], FP8, tag="p1h", bufs=2)
                p1l = ppool.tile([128, KP * 2 * NW], FP8, tag="p1l", bufs=2)
                HW = KP * NW  # half the pair-tile columns (k-pairs 0,1)
                nc.sync.dma_start(p1h[:, 0:HW], p1h_d.ap()[:, 0:HW])
                nc.sync.dma_start(p1h[:, HW:], p1h_d.ap()[:, HW:])
                nc.sync.dma_start(p1l[:, 0:HW], p1l_d.ap()[:, 0:HW])
                nc.sync.dma_start(p1l[:, HW:], p1l_d.ap()[:, HW:])
                for a in range(1, len(DN)):
                    am_dma(a)
                x0 = xp.tile([128, LT * E], F32, tag="x0")
                nc.scalar.dma_start(x0[:].rearrange("p (t d) -> p t d", t=LT),
                                    x0_d.ap().rearrange("(t p) d -> p t d", p=128))
                w0 = wp.tile([128, NW], BF16, tag="w0")
                w1 = wp.tile([16, NW], BF16, tag="w1")
                nc.scalar.dma_start(w0[:], w0_d.ap())
                nc.scalar.dma_start(w1[:], w1_d.ap())

                def am_ap(a, i, lt):
                    # DoubleRow lhsT [128, 2, 128]: k-pair i, dest l-tile lt
                    return (am[a][:]
                            .rearrange("p (j c) -> p j c", c=LH)
                            [:, 2 * i:2 * i + 2, lt * 128:(lt + 1) * 128])

                def p_ap(ph, i, col):
                    # DoubleRow rhs [128, 2, 141]
                    return (ph[:]
                            .rearrange("p (i g c) -> p i g c", i=KP, g=2)
                            [:, i, :, col:col + N1])

                # -------- gated arc aggregation for one block --------
                def arc_block(ph, pl, acc, halves=(0, 1), a_outer=True):
                    stt_i = 0
                    loops = ([(a, half) for a, _ in enumerate(DN)
                              for half in halves] if a_outer else
                             [(a, half) for half in halves
                              for a, _ in enumerate(DN)])
                    for a, half in loops:
                        d, n = DN[a]
                        col = _slab_col(d, n)
                        if True:
                            pool = (psarc, psarc2)[(a + half) % 2]
                            arc = pool.tile([128, 512], F32, tag="arc",
                                            name="arc")
                            for lt in (2 * half, 2 * half + 1):
                                off = (lt % 2) * N1
                                for i in range(KP):
                                    nc.tensor.matmul(
                                        arc[:, off:off + N1], am_ap(a, i, lt),
                                        p_ap(ph, i, col),
                                        start=(i == 0), stop=False,
                                        perf_mode=DR)
                                for i in range(KP):
                                    nc.tensor.matmul(
                                        arc[:, off:off + N1], am_ap(a, i, lt),
                                        p_ap(pl, i, col),
                                        start=False, stop=(i == KP - 1),
                                        perf_mode=DR)
                            g_sb = gpool.tile([128, 2], F32, tag="g")
                            nc.scalar.activation(
                                g_sb[:], arc[:, D:D + N1 + 1:N1],
                                mybir.ActivationFunctionType.Sigmoid)
                            for lt in (2 * half, 2 * half + 1):
                                off = (lt % 2) * N1
                                stt_i += 1
                                if stt_i % 2 == 0:
                                    nc.vector.scalar_tensor_tensor(
                                        out=acc[:, lt * D:(lt + 1) * D],
                                        in0=arc[:, off:off + D],
                                        scalar=g_sb[:, lt % 2:lt % 2 + 1],
                                        in1=acc[:, lt * D:(lt + 1) * D],
                                        op0=mybir.AluOpType.mult,
                                        op1=mybir.AluOpType.add)
                                else:
                                    # ACT applies the gate (PSUM-legal), Pool
                                    # accumulates from SBUF
                                    garc = gpool.tile([128, D], F32,
                                                      tag="garc", bufs=28,
                                                      name="garc")
                                    nc.scalar.activation(
                                        garc[:], arc[:, off:off + D],
                                        mybir.ActivationFunctionType.Copy,
                                        scale=g_sb[:, lt % 2:lt % 2 + 1])
                                    nc.gpsimd.tensor_tensor(
                                        out=acc[:, lt * D:(lt + 1) * D],
                                        in0=acc[:, lt * D:(lt + 1) * D],
                                        in1=garc[:],
                                        op=mybir.AluOpType.add)

                # ---------------- block 1 ----------------
                acc = xp.tile([128, LT * D], F32, tag="acc")
                nc.gpsimd.memset(acc[:], 0.0)
                arc_block(p1h, p1l, acc)

                # x1 = relu(acc) + x0 ; exchange natural-layout x1 (bf16)
                x1b = xp.tile([128, LT * E], BF16, tag="x1b")
                nc.vector.scalar_tensor_tensor(
                    out=x1b[:], in0=acc[:], scalar=0.0, in1=x0[:],
                    op0=mybir.AluOpType.max, op1=mybir.AluOpType.add)

                # staging + collective + loads all on gpsimd: in-order queue
                # gives the DRAM read-after-write ordering the tile framework
                # does not track for dram tensors
                nc.gpsimd.dma_start(cc_in.ap(), x1b[:])
                nc.gpsimd.collective_compute(
                    "AllGather", mybir.AluOpType.bypass,
                    replica_groups=PAIRS,
                    ins=[cc_in.ap()], outs=[cc_out.ap()])
                # x1f col layout: global l-tile ltg at col ltg*E (h-major)
                x1f = xp.tile([128, KT * E], BF16, tag="x1f")
                nc.gpsimd.dma_start(x1f[:, 0:LT * E], cc_out.ap()[0])
                nc.gpsimd.dma_start(x1f[:, LT * E:], cc_out.ap()[1])

                # full x1^T in global order: [128|12] rows x 1024 cols (+ones)
                xta = xp.tile([128, L], BF16, tag="xta")
                xtb = xp.tile([32, L], BF16, tag="xtb")
                # aligned 32-row memset; transposes overwrite rows 0..11, so
                # row 12 keeps the 1.0 bias row (rows 13+ unused)
                nc.gpsimd.memset(xtb[0:32, :], 1.0)
                for lt in range(KT):
                    tp = psmm.tile([128, 512], BF16, tag="pmm", name="tp")
                    nc.tensor.transpose(tp[:, 0:128],
                                        x1f[:, lt * E:lt * E + 128], ident[:])
                    if lt % 2 == 0:
                        nc.scalar.copy(xta[:, lt * 128:(lt + 1) * 128],
                                       tp[:, 0:128])
                    else:
                        nc.vector.tensor_copy(
                            xta[:, lt * 128:(lt + 1) * 128], tp[:, 0:128])
                    tp2 = psmm.tile([128, 512], BF16, tag="pmm", name="tp2")
                    nc.tensor.transpose(tp2[0:E - 128, 0:128],
                                        x1f[:, lt * E + 128:lt * E + E],
                                        ident[:])
                    if lt % 2 == 0:
                        nc.vector.tensor_copy(
                            xtb[0:E - 128, lt * 128:(lt + 1) * 128],
                            tp2[0:E - 128, 0:128])
                    else:
                        nc.scalar.copy(
                            xtb[0:E - 128, lt * 128:(lt + 1) * 128],
                            tp2[0:E - 128, 0:128])

                # ---------------- block-2 projections ----------------
                p2h = ppool.tile([128, KP * 2 * NW], FP8, tag="p2h")
                p2l = ppool.tile([128, KP * 2 * NW], FP8, tag="p2l")
                for m in range(KT):
                    for (c0, c1) in P_CHUNKS:
                        w = c1 - c0
                        pmm = psmm.tile([128, 512], F32, tag="pmm", name="pmm")
                        nc.tensor.matmul(pmm[:, 0:w],
                                         xta[:, m * 128:(m + 1) * 128],
                                         w0[:, c0:c1], start=True, stop=False)
                        nc.tensor.matmul(pmm[:, 0:w],
                                         xtb[0:13, m * 128:(m + 1) * 128],
                                         w1[0:13, c0:c1], start=False,
                                         stop=True)
                        dsth = p2h[:, m * NW + c0:m * NW + c1]
                        dstl = p2l[:, m * NW + c0:m * NW + c1]
                        nc.scalar.copy(dsth, pmm[:, 0:w])
                        nc.vector.tensor_tensor(
                            out=dstl, in0=pmm[:, 0:w], in1=dsth,
                            op=mybir.AluOpType.subtract)

                if DEBUG_DUMPS:
                    nc.sync.dma_start(x1o_d.ap()[rep], x1b[:])
                    nc.sync.dma_start(x1fo_d.ap()[rep], x1f[:])
                    nc.sync.dma_start(p2ho_d.ap()[rep], p2h[:])
                    nc.sync.dma_start(p2lo_d.ap()[rep], p2l[:])

                # ---------------- block 2 ----------------
                acc2 = xp.tile([128, LT * D], F32, tag="acc")
                nc.gpsimd.memset(acc2[:], 0.0)
                x2 = xp.tile([128, LT * E], F32, tag="x1")
                if DEBUG_DUMPS:
                    acc2_dump = acc2
                arc_block(p2h, p2l, acc2)
                for half in (0, 1):
                    sl = slice(half * 2 * E, (half + 1) * 2 * E)
                    nc.vector.scalar_tensor_tensor(
                        out=x2[:, sl], in0=acc2[:, sl], scalar=0.0,
                        in1=x1b[:, sl],
                        op0=mybir.AluOpType.max, op1=mybir.AluOpType.add)
                    nc.sync.dma_start(
                        out_d.ap()[rep, half * 256:(half + 1) * 256].rearrange(
                            "(t p) d -> p t d", p=128),
                        x2[:, sl].rearrange("p (t d) -> p t d", t=2))
                    if DEBUG_DUMPS:
                        nc.sync.dma_start(
                            acc2o_d.ap()[rep][:, half * 2 * D:(half + 1) * 2 * D],
                            acc2[:, half * 2 * D:(half + 1) * 2 * D])

    nc.compile()
    return nc


def _get_nc():
    global _NC
    if _NC is None:
        _NC = _build()
    return _NC


def _prep_inputs(seq_repr, adj, W_in, b_in, W_out, b_out,
                 Wg_in, bg_in, Wg_out, bg_out):
    """Host-side sharding + layout prep for the 8 per-core input maps."""
    seq_repr = np.asarray(seq_repr, np.float32)
    adj = np.asarray(adj)
    W_in = np.asarray(W_in, np.float32); b_in = np.asarray(b_in, np.float32)
    W_out = np.asarray(W_out, np.float32); b_out = np.asarray(b_out, np.float32)
    Wg_in = np.asarray(Wg_in, np.float32); bg_in = np.asarray(bg_in, np.float32)
    Wg_out = np.asarray(Wg_out, np.float32); bg_out = np.asarray(bg_out, np.float32)

    # ---- block-2 weight slabs (shared by all cores) ----
    w0 = np.zeros((128, NW), np.float32)
    w1 = np.zeros((16, NW), np.float32)
    for d in range(2):
        Wd, bd = (W_in, b_in) if d == 0 else (W_out, b_out)
        Wgd, bgd = (Wg_in, bg_in) if d == 0 else (Wg_out, bg_out)
        for s in range(ND):
            c = (d * ND + s) * N1
            w0[:, c:c + D] = Wd[1, s][0:128]
            w1[0:12, c:c + D] = Wd[1, s][128:E]
            w1[12, c:c + D] = bd[1, s]
            w0[:, c + D] = Wgd[1, s][0:128, 0]
            w1[0:12, c + D] = Wgd[1, s][128:E, 0]
            w1[12, c + D] = bgd[1, s, 0]
    # x0.25: adjacency is scaled x4 (exact in fp8); keeps |p2| well under
    # the TRN2 fp8e4 saturation point (+-240)
    w0 = (0.25 * w0).astype(BF)
    w1 = (0.25 * w1).astype(BF)

    # ---- block-1 projections, host-computed, fp8 hi/lo pair tiles ----
    p1h_by_b, p1l_by_b = [], []
    for b in range(B):
        x = seq_repr[b]                       # (L, E)
        P = np.zeros((L, NW), np.float32)
        for d in range(2):
            Wd, bd = (W_in, b_in) if d == 0 else (W_out, b_out)
            Wgd, bgd = (Wg_in, bg_in) if d == 0 else (Wg_out, bg_out)
            for s in range(ND):
                c = (d * ND + s) * N1
                P[:, c:c + D] = x @ Wd[0, s] + bd[0, s]
                P[:, c + D] = x @ Wgd[0, s][:, 0] + bgd[0, s, 0]
        P *= 0.25            # compensates the x4-scaled adjacency
        Ph = P.astype(E4)
        Pl = (P - Ph.astype(np.float32)).astype(E4)
        # pair-tile layout [128, KT*NW]: t[p, m*NW + c] = P[m*128 + p, c]
        ph = np.ascontiguousarray(
            Ph.reshape(KT, 128, NW).transpose(1, 0, 2).reshape(128, KT * NW))
        pl = np.ascontiguousarray(
            Pl.reshape(KT, 128, NW).transpose(1, 0, 2).reshape(128, KT * NW))
        p1h_by_b.append(ph)
        p1l_by_b.append(pl)

    in_maps = []
    for c in range(NCORES):
        b, h = c // 2, c % 2
        sl = slice(h * LH, (h + 1) * LH)
        a = (adj[b] * 4).astype(np.int8)      # 0/4: exact in fp8e4
        am = np.empty((2 * NE, L, LH), E4)
        for n in range(NE):
            # in-arcs: lhsT[m, l] = A[l, m] for own dest rows l
            am[2 * n] = a[n][sl, :].T.astype(E4)
            # out-arcs: lhsT[m, l] = A^T[l, m] = A[m, l]
            am[2 * n + 1] = a[n][:, sl].astype(E4)
        in_maps.append({
            "am": am,
            "p1h": p1h_by_b[b], "p1l": p1l_by_b[b],
            "x0": np.ascontiguousarray(seq_repr[b][sl]),
            "w0": w0, "w1": w1,
        })
    return in_maps


def _combine(results):
    out = np.empty((B, L, E), np.float32)
    for b in range(B):
        out[b, 0:LH] = results[2 * b]["outp"][0]
        out[b, LH:L] = results[2 * b + 1]["outp"][0]
    return out


def run_on_hw(in_maps, trace=False, **kw):
    nc = _get_nc()
    res = run_bass_kernel_spmd(nc, in_maps, core_ids=list(range(NCORES)),
                               trace=trace, **kw)
    return res


def kernel(**inputs):
    in_maps = _prep_inputs(**inputs)
    res = run_on_hw(in_maps)
    return _combine(res.results)


# revision 58
# speedup vs baseline: 1.0308x; 1.0089x over previous
"""Bass/Trainium2 kernel for nn_GCNN_61615600828570 (gated GCNN message passing).

Self-contained: hardcodes shapes/sharding. 8 NeuronCores, sharded as
(batch b, l-half h) — each core computes BOTH arc directions and all 10 edge
types for its 512 destination rows. A single pair AllGather exchanges the
transposed block-1 output x1^T between the two GCN blocks.

Key device-side choices:
- adjacency in fp8e4 (0/1 exact), all 20 (dir,type) slabs SBUF-resident
- arc matmuls in fp8e4 DoubleRow perf mode (2 k-slabs per instruction,
  0.5 cycles/row) with hi+lo quantization compensation of the projections
  (p = p_hi + p_lo, both e4m3; error ~1e-3)
- block-1 projections (x0 @ W + b) precomputed host-side straight into the
  fp8 hi/lo pair-tile layout; block-2 projections computed on device from
  the gathered x1^T, quantized to hi/lo on ACT/DVE.

kernel(**inputs) takes the FULL inputs (numpy, dtypes as in setup_inputs)
and returns the FULL (B, L, E) float32 output.
"""
import numpy as np
import ml_dtypes

import concourse.bass as bass
import concourse.mybir as mybir
import concourse.tile as tile
from concourse import bacc
from concourse.bass_utils import run_bass_kernel_spmd
from concourse.masks import make_identity

F32 = mybir.dt.float32
BF16 = mybir.dt.bfloat16
FP8 = mybir.dt.float8e4
BF = ml_dtypes.bfloat16
E4 = ml_dtypes.float8_e4m3fn
DR = mybir.MatmulPerfMode.DoubleRow

B, L, E, D = 4, 1024, 140, 140
NE, NU, NB = 10, 4, 2
ND = NU + 1
N1 = D + 1            # 141: D outputs + gate column
LH = L // 2           # 512 destination rows per core
LT = LH // 128        # 4 l-tiles per core
KT = L // 128         # 8 contraction k-tiles
KP = KT // 2          # 4 DoubleRow k-pair tiles
NCORES = 8
PAIRS = [[0, 1], [2, 3], [4, 5], [6, 7]]
NW = 2 * ND * N1      # 1410: in-sets 0..704, out-sets 705..1409
ET = [min(n, NU) for n in range(NE)]
# (dir, n) pair list; dir 0 = in-arcs (A), dir 1 = out-arcs (A^T)
DN = [(d, n) for n in range(NE) for d in (0, 1)]
P_CHUNKS = [(0, 512), (512, 1024), (1024, NW)]
XC = E * LH           # exchanged x1^T elements per core (140 rows x 512)

_NC = None
DEBUG_DUMPS = False


def _slab_col(d, n):
    return (ET[n] if d == 0 else ND + ET[n]) * N1


def _build(reps=1):
    nc = bacc.Bacc("TRN2", target_bir_lowering=False, debug=False,
                   num_devices=NCORES)

    # lhsT A slabs, one per (dir, n): [k=1024 global-m rows, 512 own-dest cols]
    am_d = nc.dram_tensor("am", [2 * NE, L, LH], FP8, kind="ExternalInput")
    # block-1 projections, host-quantized hi/lo pair tiles [128, KP*2*NW]
    p1h_d = nc.dram_tensor("p1h", [128, KP * 2 * NW], FP8, kind="ExternalInput")
    p1l_d = nc.dram_tensor("p1l", [128, KP * 2 * NW], FP8, kind="ExternalInput")
    x0_d = nc.dram_tensor("x0", [LH, E], F32, kind="ExternalInput")
    w0_d = nc.dram_tensor("w0", [128, NW], BF16, kind="ExternalInput")
    w1_d = nc.dram_tensor("w1", [16, NW], BF16, kind="ExternalInput")

    out_d = nc.dram_tensor("outp", [reps, LH, E], F32, kind="ExternalOutput")
    if DEBUG_DUMPS:
        x1o_d = nc.dram_tensor("x1o", [reps, 128, LT * E], BF16,
                               kind="ExternalOutput")
        x1fo_d = nc.dram_tensor("x1fo", [reps, 128, KT * E], BF16,
                                kind="ExternalOutput")
        p2ho_d = nc.dram_tensor("p2ho", [reps, 128, KT * NW], FP8,
                                kind="ExternalOutput")
        p2lo_d = nc.dram_tensor("p2lo", [reps, 128, KT * NW], FP8,
                                kind="ExternalOutput")
        acc2o_d = nc.dram_tensor("acc2o", [reps, 128, LT * D], F32,
                                 kind="ExternalOutput")


    # partition-major exchange buffers: straight [128, 560] SBUF<->DRAM copies
    cc_in = nc.dram_tensor("cc_in", [128, LT * E], BF16)
    cc_out = nc.dram_tensor("cc_out", [2, 128, LT * E], BF16)

    with tile.TileContext(nc) as tc:
        with (
            tc.tile_pool(name="cst", bufs=1) as cst,
            tc.tile_pool(name="amp", bufs=2 * NE) as amp,
            tc.tile_pool(name="pp", bufs=1) as ppool,
            tc.tile_pool(name="xp", bufs=2) as xp,
            tc.tile_pool(name="wp", bufs=1) as wp,
            tc.tile_pool(name="gp", bufs=80) as gpool,
            tc.tile_pool(name="psarc", bufs=3, space="PSUM") as psarc,
            tc.tile_pool(name="psarc2", bufs=2, space="PSUM") as psarc2,
            tc.tile_pool(name="psmm", bufs=3, space="PSUM") as psmm,
        ):
            ident = cst.tile([128, 128], BF16)
            make_identity(nc, ident[:])

            am_view = am_d.ap().rearrange("a (j p) c -> a p j c", p=128)

            for rep in range(reps):
                # ---------------- phase 0: DMAs ----------------
                # single sync queue in exact consumption order: am0 first so
                # the PE can start, then the block-1 projections, then the
                # remaining A slabs; x0/w on the scalar queue (needed late)
                am = [amp.tile([128, KT * LH], FP8, tag="am", name=f"am{a}")
                      for a in range(len(DN))]

                def am_dma(a):
                    d, n = DN[a]
                    nc.sync.dma_start(
                        am[a][:].rearrange("p (j c) -> p j c", c=LH),
                        am_view[2 * n + d])

                am_dma(0)
                p1h = ppool.tile([128, KP * 2 * NW], FP8, tag="p1h", bufs=1)
                p1l = ppool.tile([128, KP * 2 * NW], FP8, tag="p1l", bufs=1)
                HW = KP * NW  # half the pair-tile columns (k-pairs 0,1)
                nc.sync.dma_start(p1h[:, 0:HW], p1h_d.ap()[:, 0:HW])
                nc.sync.dma_start(p1h[:, HW:], p1h_d.ap()[:, HW:])
                nc.sync.dma_start(p1l[:, 0:HW], p1l_d.ap()[:, 0:HW])
                nc.sync.dma_start(p1l[:, HW:], p1l_d.ap()[:, HW:])
                for a in range(1, len(DN)):
                    am_dma(a)
                x0 = xp.tile([128, LT * E], F32, tag="x0")
                nc.scalar.dma_start(x0[:].rearrange("p (t d) -> p t d", t=LT),
                                    x0_d.ap().rearrange("(t p) d -> p t d", p=128))
                w0 = wp.tile([128, NW], BF16, tag="w0")
                w1 = wp.tile([16, NW], BF16, tag="w1")
                nc.scalar.dma_start(w0[:], w0_d.ap())
                nc.scalar.dma_start(w1[:], w1_d.ap())

                def am_ap(a, i, lt):
                    # DoubleRow lhsT [128, 2, 128]: k-pair i, dest l-tile lt
                    return (am[a][:]
                            .rearrange("p (j c) -> p j c", c=LH)
                            [:, 2 * i:2 * i + 2, lt * 128:(lt + 1) * 128])

                def p_ap(ph, i, col):
                    # DoubleRow rhs [128, 2, 141]
                    return (ph[:]
                            .rearrange("p (i g c) -> p i g c", i=KP, g=2)
                            [:, i, :, col:col + N1])

                # -------- gated arc aggregation for one block --------
                def arc_block(ph, pl, acc, halves=(0, 1), a_outer=True):
                    stt_i = 0
                    loops = ([(a, half) for a, _ in enumerate(DN)
                              for half in halves] if a_outer else
                             [(a, half) for half in halves
                              for a, _ in enumerate(DN)])
                    for a, half in loops:
                        d, n = DN[a]
                        col = _slab_col(d, n)
                        if True:
                            pool = (psarc, psarc2)[(a + half) % 2]
                            arc = pool.tile([128, 512], F32, tag="arc",
                                            name="arc")
                            for lt in (2 * half, 2 * half + 1):
                                off = (lt % 2) * N1
                                for i in range(KP):
                                    nc.tensor.matmul(
                                        arc[:, off:off + N1], am_ap(a, i, lt),
                                        p_ap(ph, i, col),
                                        start=(i == 0), stop=False,
                                        perf_mode=DR)
                                for i in range(KP):
                                    nc.tensor.matmul(
                                        arc[:, off:off + N1], am_ap(a, i, lt),
                                        p_ap(pl, i, col),
                                        start=False, stop=(i == KP - 1),
                                        perf_mode=DR)
                            g_sb = gpool.tile([128, 2], F32, tag="g")
                            nc.scalar.activation(
                                g_sb[:], arc[:, D:D + N1 + 1:N1],
                                mybir.ActivationFunctionType.Sigmoid)
                            for lt in (2 * half, 2 * half + 1):
                                off = (lt % 2) * N1
                                stt_i += 1
                                if stt_i % 2 == 0:
                                    nc.vector.scalar_tensor_tensor(
                                        out=acc[:, lt * D:(lt + 1) * D],
                                        in0=arc[:, off:off + D],
                                        scalar=g_sb[:, lt % 2:lt % 2 + 1],
                                        in1=acc[:, lt * D:(lt + 1) * D],
                                        op0=mybir.AluOpType.mult,
                                        op1=mybir.AluOpType.add)
                                else:
                                    # ACT applies the gate (PSUM-legal), Pool
                                    # accumulates from SBUF
                                    garc = gpool.tile([128, D], F32,
                                                      tag="garc", bufs=80,
                                                      name="garc")
                                    nc.scalar.activation(
                                        garc[:], arc[:, off:off + D],
                                        mybir.ActivationFunctionType.Copy,
                                        scale=g_sb[:, lt % 2:lt % 2 + 1])
                                    nc.gpsimd.tensor_tensor(
                                        out=acc[:, lt * D:(lt + 1) * D],
                                        in0=acc[:, lt * D:(lt + 1) * D],
                                        in1=garc[:],
                                        op=mybir.AluOpType.add)

                # ---------------- block 1 ----------------
                acc = xp.tile([128, LT * D], F32, tag="acc")
                nc.gpsimd.memset(acc[:], 0.0)
                arc_block(p1h, p1l, acc)

                # x1 = relu(acc) + x0 ; exchange natural-layout x1 (bf16)
                x1b = xp.tile([128, LT * E], BF16, tag="x1b")
                nc.vector.scalar_tensor_tensor(
                    out=x1b[:], in0=acc[:], scalar=0.0, in1=x0[:],
                    op0=mybir.AluOpType.max, op1=mybir.AluOpType.add)

                # staging + collective + loads all on gpsimd: in-order queue
                # gives the DRAM read-after-write ordering the tile framework
                # does not track for dram tensors
                nc.gpsimd.dma_start(cc_in.ap(), x1b[:])
                nc.gpsimd.collective_compute(
                    "AllGather", mybir.AluOpType.bypass,
                    replica_groups=PAIRS,
                    ins=[cc_in.ap()], outs=[cc_out.ap()])
                # x1f col layout: global l-tile ltg at col ltg*E (h-major)
                x1f = xp.tile([128, KT * E], BF16, tag="x1f")
                nc.gpsimd.dma_start(x1f[:, 0:LT * E], cc_out.ap()[0])
                nc.gpsimd.dma_start(x1f[:, LT * E:], cc_out.ap()[1])

                # full x1^T in global order: [128|12] rows x 1024 cols (+ones)
                xta = xp.tile([128, L], BF16, tag="xta")
                xtb = xp.tile([32, L], BF16, tag="xtb")
                # aligned 32-row memset; transposes overwrite rows 0..11, so
                # row 12 keeps the 1.0 bias row (rows 13+ unused)
                nc.gpsimd.memset(xtb[0:32, :], 1.0)
                for lt in range(KT):
                    tp = psmm.tile([128, 512], BF16, tag="pmm", name="tp")
                    nc.tensor.transpose(tp[:, 0:128],
                                        x1f[:, lt * E:lt * E + 128], ident[:])
                    if lt % 2 == 0:
                        nc.scalar.copy(xta[:, lt * 128:(lt + 1) * 128],
                                       tp[:, 0:128])
                    else:
                        nc.vector.tensor_copy(
                            xta[:, lt * 128:(lt + 1) * 128], tp[:, 0:128])
                    tp2 = psmm.tile([128, 512], BF16, tag="pmm", name="tp2")
                    nc.tensor.transpose(tp2[0:E - 128, 0:128],
                                        x1f[:, lt * E + 128:lt * E + E],
                                        ident[:])
                    if lt % 2 == 0:
                        nc.vector.tensor_copy(
                            xtb[0:E - 128, lt * 128:(lt + 1) * 128],
                            tp2[0:E - 128, 0:128])
                    else:
                        nc.scalar.copy(
                            xtb[0:E - 128, lt * 128:(lt + 1) * 128],
                            tp2[0:E - 128, 0:128])

                # ---------------- block-2 projections ----------------
                p2h = ppool.tile([128, KP * 2 * NW], FP8, tag="p2h")
                p2l = ppool.tile([128, KP * 2 * NW], FP8, tag="p2l")
                for m in range(KT):
                    for (c0, c1) in P_CHUNKS:
                        w = c1 - c0
                        pmm = psmm.tile([128, 512], F32, tag="pmm", name="pmm")
                        nc.tensor.matmul(pmm[:, 0:w],
                                         xta[:, m * 128:(m + 1) * 128],
                                         w0[:, c0:c1], start=True, stop=False)
                        nc.tensor.matmul(pmm[:, 0:w],
                                         xtb[0:13, m * 128:(m + 1) * 128],
                                         w1[0:13, c0:c1], start=False,
                                         stop=True)
                        dsth = p2h[:, m * NW + c0:m * NW + c1]
                        dstl = p2l[:, m * NW + c0:m * NW + c1]
                        nc.scalar.copy(dsth, pmm[:, 0:w])
                        nc.vector.tensor_tensor(
                            out=dstl, in0=pmm[:, 0:w], in1=dsth,
                            op=mybir.AluOpType.subtract)

                if DEBUG_DUMPS:
                    nc.sync.dma_start(x1o_d.ap()[rep], x1b[:])
                    nc.sync.dma_start(x1fo_d.ap()[rep], x1f[:])
                    nc.sync.dma_start(p2ho_d.ap()[rep], p2h[:])
                    nc.sync.dma_start(p2lo_d.ap()[rep], p2l[:])

                # ---------------- block 2 ----------------
                acc2 = xp.tile([128, LT * D], F32, tag="acc")
                nc.gpsimd.memset(acc2[:], 0.0)
                x2 = xp.tile([128, LT * E], F32, tag="x1")
                if DEBUG_DUMPS:
                    acc2_dump = acc2
                arc_block(p2h, p2l, acc2)
                for half in (0, 1):
                    sl = slice(half * 2 * E, (half + 1) * 2 * E)
                    nc.vector.scalar_tensor_tensor(
                        out=x2[:, sl], in0=acc2[:, sl], scalar=0.0,
                        in1=x1b[:, sl],
                        op0=mybir.AluOpType.max, op1=mybir.AluOpType.add)
                    nc.sync.dma_start(
                        out_d.ap()[rep, half * 256:(half + 1) * 256].rearrange(
                            "(t p) d -> p t d", p=128),
                        x2[:, sl].rearrange("p (t d) -> p t d", t=2))
                    if DEBUG_DUMPS:
                        nc.sync.dma_start(
                            acc2o_d.ap()[rep][:, half * 2 * D:(half + 1) * 2 * D],
                            acc2[:, half * 2 * D:(half + 1) * 2 * D])

    nc.compile()
    return nc


def _get_nc():
    global _NC
    if _NC is None:
        _NC = _build()
    return _NC


def _prep_inputs(seq_repr, adj, W_in, b_in, W_out, b_out,
                 Wg_in, bg_in, Wg_out, bg_out):
    """Host-side sharding + layout prep for the 8 per-core input maps."""
    seq_repr = np.asarray(seq_repr, np.float32)
    adj = np.asarray(adj)
    W_in = np.asarray(W_in, np.float32); b_in = np.asarray(b_in, np.float32)
    W_out = np.asarray(W_out, np.float32); b_out = np.asarray(b_out, np.float32)
    Wg_in = np.asarray(Wg_in, np.float32); bg_in = np.asarray(bg_in, np.float32)
    Wg_out = np.asarray(Wg_out, np.float32); bg_out = np.asarray(bg_out, np.float32)

    # ---- block-2 weight slabs (shared by all cores) ----
    w0 = np.zeros((128, NW), np.float32)
    w1 = np.zeros((16, NW), np.float32)
    for d in range(2):
        Wd, bd = (W_in, b_in) if d == 0 else (W_out, b_out)
        Wgd, bgd = (Wg_in, bg_in) if d == 0 else (Wg_out, bg_out)
        for s in range(ND):
            c = (d * ND + s) * N1
            w0[:, c:c + D] = Wd[1, s][0:128]
            w1[0:12, c:c + D] = Wd[1, s][128:E]
            w1[12, c:c + D] = bd[1, s]
            w0[:, c + D] = Wgd[1, s][0:128, 0]
            w1[0:12, c + D] = Wgd[1, s][128:E, 0]
            w1[12, c + D] = bgd[1, s, 0]
    # x0.25: adjacency is scaled x4 (exact in fp8); keeps |p2| well under
    # the TRN2 fp8e4 saturation point (+-240)
    w0 = (0.25 * w0).astype(BF)
    w1 = (0.25 * w1).astype(BF)

    # ---- block-1 projections, host-computed, fp8 hi/lo pair tiles ----
    p1h_by_b, p1l_by_b = [], []
    for b in range(B):
        x = seq_repr[b]                       # (L, E)
        P = np.zeros((L, NW), np.float32)
        for d in range(2):
            Wd, bd = (W_in, b_in) if d == 0 else (W_out, b_out)
            Wgd, bgd = (Wg_in, bg_in) if d == 0 else (Wg_out, bg_out)
            for s in range(ND):
                c = (d * ND + s) * N1
                P[:, c:c + D] = x @ Wd[0, s] + bd[0, s]
                P[:, c + D] = x @ Wgd[0, s][:, 0] + bgd[0, s, 0]
        P *= 0.25            # compensates the x4-scaled adjacency
        Ph = P.astype(E4)
        Pl = (P - Ph.astype(np.float32)).astype(E4)
        # pair-tile layout [128, KT*NW]: t[p, m*NW + c] = P[m*128 + p, c]
        ph = np.ascontiguousarray(
            Ph.reshape(KT, 128, NW).transpose(1, 0, 2).reshape(128, KT * NW))
        pl = np.ascontiguousarray(
            Pl.reshape(KT, 128, NW).transpose(1, 0, 2).reshape(128, KT * NW))
        p1h_by_b.append(ph)
        p1l_by_b.append(pl)

    in_maps = []
    for c in range(NCORES):
        b, h = c // 2, c % 2
        sl = slice(h * LH, (h + 1) * LH)
        a = (adj[b] * 4).astype(np.int8)      # 0/4: exact in fp8e4
        am = np.empty((2 * NE, L, LH), E4)
        for n in range(NE):
            # in-arcs: lhsT[m, l] = A[l, m] for own dest rows l
            am[2 * n] = a[n][sl, :].T.astype(E4)
            # out-arcs: lhsT[m, l] = A^T[l, m] = A[m, l]
            am[2 * n + 1] = a[n][:, sl].astype(E4)
        in_maps.append({
            "am": am,
            "p1h": p1h_by_b[b], "p1l": p1l_by_b[b],
            "x0": np.ascontiguousarray(seq_repr[b][sl]),
            "w0": w0, "w1": w1,
        })
    return in_maps


def _combine(results):
    out = np.empty((B, L, E), np.float32)
    for b in range(B):
        out[b, 0:LH] = results[2 * b]["outp"][0]
        out[b, LH:L] = results[2 * b + 1]["outp"][0]
    return out


def run_on_hw(in_maps, trace=False, **kw):
    nc = _get_nc()
    res = run_bass_kernel_spmd(nc, in_maps, core_ids=list(range(NCORES)),
                               trace=trace, **kw)
    return res


def kernel(**inputs):
    in_maps = _prep_inputs(**inputs)
    res = run_on_hw(in_maps)
    return _combine(res.results)
